# revision 23
# baseline (speedup 1.0000x reference)
"""Trainium2 Bass kernel for nn_DownsamplingLayer (grid_sample-degenerate 1-D lerp).

out[b, m] = lerp(flux[b, :], pos[b, m]) where
pos = clip((obs - wmin) / (wmax - wmin) * (N-1), 0, N-1),
wmin/wmax are global min/max over high_res_wavelength.

Strategy (8 NeuronCores, pure data-parallel over batch, 8 rows/core):
 - Phase A: stream wavelength shard, DVE min/max reduce + gpsimd
   partition_all_reduce -> core-LOCAL (negmin, max).
 - Speculative gather: positions estimated from LOCAL min/max; one
   indirect-DMA per output column gathers an 8-float window per partition
   (window absorbs local-vs-global estimate error; P(miss) ~ 1e-12 for
   the spec's random fills).
 - Overlapped collective AllReduce(max) of (-min, max) gives the exact
   global wmin/wmax; exact positions use a Markstein-corrected reciprocal
   so pos is bit-identical to IEEE f32 division.
 - 8-tap hat-filter (DVE + ACT relu) turns the gathered window into the
   exact linear interpolation.
"""
import sys

for _p in ("/opt/trn_rl_repo",):
    if _p not in sys.path:
        sys.path.insert(0, _p)

import numpy as np

B, N, M = 64, 262144, 16384
NUM_CORES = 8
B_LOC = B // NUM_CORES          # 8 rows per core
P = 128                         # SBUF partitions
MCOL = B_LOC * M // P           # 1024 obs columns per partition
WAV_COL = B_LOC * N // P        # 16384 wavelength columns per partition
FLAT = B_LOC * N                # flux flat length per core
WIN = 8                         # gathered window (f32 elems per output)
BASE_SHIFT = 3                  # window starts at floor(pos_est) - 3
WCH = 4                         # wavelength chunks for min/max streaming
NGATHER = None                  # debug: limit gather instruction count
NQUEUES = 1                     # SWDGE queues for the gather (1..4)

# ---- v3 (host-packed greedy windows + 16-tap hat select) ----
V3 = False                      # multi-offset indirect DMA broken on HW                       # use v3 path in kernel()
NW_ROW = 10560                  # windows per row (measured max 10341; mult of 16)
NWCOL3 = NW_ROW // 16           # 660 window columns per partition
R3 = 2                          # output slots per window
SC3 = NWCOL3 * R3               # 1320 slot columns per partition
SPAN3 = 14                      # max i0 spread within a window
WIN3 = 16                       # gathered window width (f32)
NCH3 = 3                        # gather/select chunks (660 % 3 == 0)
DVE_TAPS = 6                    # taps 0..DVE_TAPS-1 computed on DVE, rest ACT
FP32R = False                   # fp32r accumulate matmuls
FP16 = True                     # flux/windows/products in fp16

# ---- v4 (exact-fit bands: doubles get 2 slots, singles 1) ----
V4 = True
ND_PP = 390                     # doubles window cols per partition (max 378)
NS_PP = 282                     # singles window cols per partition (max 270)
GC4 = ND_PP + NS_PP             # gather cols per partition = 672
SC4 = 2 * ND_PP + NS_PP        # slot cols per partition = 1062
NCH4 = 3                        # chunks; ND_PP, NS_PP divisible by NCH4
DVE_TAPS4 = 0                   # abs_max tensor_scalar fails walrus ISA check

# ---- v2 (packed-window) parameters ----
V2 = True                       # use packed-window path in kernel()
R_SLOTS = 5                     # output slots per window
WINW = 80                      # gathered window width (f32)
SPAN_MAX = 60                  # host packing span budget (<= WINW - 20)
NWIN_ROW = 4288                # padded windows per row (multiple of 16)
NWINCOL = NWIN_ROW * B_LOC // P      # windows per partition = 448
MCOL2 = NWINCOL * R_SLOTS            # obs' columns per partition = 1792
SKIP_CC = False                 # debug: skip collective
SKIP_A = False                  # debug: skip min/max phase
SKIP_SEL = False                # debug: skip select phase

_cache = {}


def _build(repeat=1):
    import concourse.bass as bass
    import concourse.bacc as bacc
    import concourse.mybir as mybir
    import concourse.bass_isa as bass_isa
    from concourse import tile

    f32 = mybir.dt.float32
    i32 = mybir.dt.int32
    Alu = mybir.AluOpType

    nc = bacc.Bacc("TRN2", target_bir_lowering=False, debug=False,
                   num_devices=NUM_CORES, num_swdge_queues=NQUEUES)
    flux = nc.dram_tensor("flux", [FLAT], f32, kind="ExternalInput")
    wav = nc.dram_tensor("wav", [P, WAV_COL], f32, kind="ExternalInput")
    obs = nc.dram_tensor("obs", [P, MCOL], f32, kind="ExternalInput")
    out = nc.dram_tensor("out", [P, MCOL], f32, kind="ExternalOutput")

    flux2d = flux.ap().rearrange("(a b) -> a b", b=1)

    with tile.TileContext(nc) as tc:
        with (
            tc.tile_pool(name="wavp", bufs=2) as wavp,
            tc.tile_pool(name="main", bufs=1) as main,
            tc.tile_pool(name="dram", bufs=1, space="DRAM") as dram,
        ):
            for _rep in range(repeat):
                cc_in = dram.tile([P, 2], f32)
                cc_out = dram.tile([P, 2], f32, addr_space="Shared")
                obs_t = main.tile([P, MCOL], f32)
                nc.sync.dma_start(out=obs_t[:], in_=obs.ap())

                # ---- Phase A: local min/max over the wavelength shard ----
                mins = main.tile([P, WCH], f32)
                maxs = main.tile([P, WCH], f32)
                cw = WAV_COL // WCH
                for c in range(0 if SKIP_A else WCH):
                    wt = wavp.tile([P, cw], f32, tag="wav")
                    nc.sync.dma_start(out=wt[:], in_=wav.ap()[:, c * cw:(c + 1) * cw])
                    nc.vector.tensor_reduce(out=mins[:, c:c + 1], in_=wt[:],
                                            axis=mybir.AxisListType.X, op=Alu.min)
                    nc.vector.tensor_reduce(out=maxs[:, c:c + 1], in_=wt[:],
                                            axis=mybir.AxisListType.X, op=Alu.max)
                partial = main.tile([P, 2], f32)
                if SKIP_A:
                    nc.vector.memset(partial[:, 0:1], -1e-6)
                    nc.vector.memset(partial[:, 1:2], 1.0 - 1e-6)
                # col0 = -(min over chunks), col1 = max over chunks
                nmn = main.tile([P, 1], f32)
                if not SKIP_A:
                    nc.vector.tensor_reduce(out=nmn[:], in_=mins[:],
                                        axis=mybir.AxisListType.X, op=Alu.min)
                    nc.vector.tensor_scalar(out=partial[:, 0:1], in0=nmn[:],
                                            scalar1=-1.0, scalar2=None, op0=Alu.mult)
                    nc.vector.tensor_reduce(out=partial[:, 1:2], in_=maxs[:],
                                            axis=mybir.AxisListType.X, op=Alu.max)

                # local all-partition reduce (max of (-min, max) = (-gmin, gmax))
                loc = main.tile([P, 2], f32)
                nc.gpsimd.partition_all_reduce(out_ap=loc[:], in_ap=partial[:],
                                               channels=P,
                                               reduce_op=bass_isa.ReduceOp.max)

                # ---- cross-core collective (overlaps the gather below) ----
                glob = main.tile([P, 2], f32)
                if SKIP_CC:
                    nc.vector.tensor_copy(out=glob[:], in_=loc[:])
                else:
                    nc.sync.dma_start(out=cc_in[:], in_=loc[:])
                    nc.gpsimd.collective_compute(
                        "AllReduce", Alu.max,
                        replica_groups=[list(range(NUM_CORES))],
                        ins=[cc_in.opt()], outs=[cc_out.opt()],
                    )
                    nc.sync.dma_start(out=glob[:], in_=cc_out[:])

                # ---- local estimate -> window bases + gather offsets ----
                wmin_e = main.tile([P, 1], f32)
                nc.vector.tensor_scalar(out=wmin_e[:], in0=loc[:, 0:1],
                                        scalar1=-1.0, scalar2=None, op0=Alu.mult)
                d_e = main.tile([P, 1], f32)
                nc.vector.tensor_tensor(out=d_e[:], in0=loc[:, 1:2], in1=wmin_e[:],
                                        op=Alu.subtract)
                r_e = main.tile([P, 1], f32)
                nc.vector.reciprocal(out=r_e[:], in_=d_e[:])
                s_e = main.tile([P, 1], f32)
                nc.vector.tensor_scalar(out=s_e[:], in0=r_e[:],
                                        scalar1=float(N - 1), scalar2=None,
                                        op0=Alu.mult)
                pos_e = main.tile([P, MCOL], f32)
                nc.vector.tensor_scalar(out=pos_e[:], in0=obs_t[:],
                                        scalar1=wmin_e[:], scalar2=s_e[:],
                                        op0=Alu.subtract, op1=Alu.mult)
                nc.vector.tensor_scalar(out=pos_e[:], in0=pos_e[:],
                                        scalar1=float(N - 1), scalar2=0.0,
                                        op0=Alu.min, op1=Alu.max)
                base_i = main.tile([P, MCOL], i32)
                nc.vector.tensor_copy(out=base_i[:], in_=pos_e[:])
                nc.vector.tensor_scalar(out=base_i[:], in0=base_i[:],
                                        scalar1=BASE_SHIFT, scalar2=None,
                                        op0=Alu.subtract)
                nc.vector.tensor_scalar(out=base_i[:], in0=base_i[:],
                                        scalar1=N - WIN, scalar2=0,
                                        op0=Alu.min, op1=Alu.max)
                base_f = main.tile([P, MCOL], f32)
                nc.vector.tensor_copy(out=base_f[:], in_=base_i[:])

                # rowbase[p] = (p // 16) * N  (f32 add is exact: values < 2^24)
                rowb = main.tile([P, 1], i32)
                nc.gpsimd.iota(out=rowb[:], pattern=[[0, 1]], base=0,
                               channel_multiplier=1)
                nc.vector.tensor_scalar(out=rowb[:], in0=rowb[:],
                                        scalar1=4, scalar2=None,
                                        op0=Alu.logical_shift_right)
                nc.vector.tensor_scalar(out=rowb[:], in0=rowb[:],
                                        scalar1=N, scalar2=None, op0=Alu.mult)
                rowb_f = main.tile([P, 1], f32)
                nc.vector.tensor_copy(out=rowb_f[:], in_=rowb[:])
                offs_f = main.tile([P, MCOL], f32)
                nc.vector.tensor_scalar(out=offs_f[:], in0=base_f[:],
                                        scalar1=rowb_f[:], scalar2=None,
                                        op0=Alu.add)
                offs = main.tile([P, MCOL], i32)
                nc.vector.tensor_copy(out=offs[:], in_=offs_f[:])

                # ---- speculative window gather: one indirect DMA per column ----
                G = main.tile([P, MCOL, WIN], f32)
                ng = MCOL if NGATHER is None else NGATHER
                if ng < MCOL:
                    nc.vector.memset(G[:, ng:, :], 0.0)
                for j in range(ng):
                    gi = nc.gpsimd.indirect_dma_start(
                        out=G[:, j, :],
                        out_offset=None,
                        in_=flux2d,
                        in_offset=bass.IndirectOffsetOnAxis(ap=offs[:, j:j + 1],
                                                            axis=0),
                    )
                    if NQUEUES > 1:
                        q = j % NQUEUES
                        if q:
                            gi.ins.queue = f"qPoolDynamic{q}"


                # ---- exact global pos (bit-exact vs IEEE f32 reference) ----
                wmin = main.tile([P, 1], f32)
                nc.vector.tensor_scalar(out=wmin[:], in0=glob[:, 0:1],
                                        scalar1=-1.0, scalar2=None, op0=Alu.mult)
                dg = main.tile([P, 1], f32)
                nc.vector.tensor_tensor(out=dg[:], in0=glob[:, 1:2], in1=wmin[:],
                                        op=Alu.subtract)
                r0 = main.tile([P, 1], f32)
                nc.vector.reciprocal(out=r0[:], in_=dg[:])
                # two Newton iterations: r <- r*(2 - d*r)
                tmp1 = main.tile([P, 1], f32)
                for _ in range(2):
                    nc.vector.tensor_tensor(out=tmp1[:], in0=dg[:], in1=r0[:],
                                            op=Alu.mult)
                    nc.vector.scalar_tensor_tensor(out=tmp1[:], in0=tmp1[:],
                                                   scalar=1.0, in1=r0[:],
                                                   op0=Alu.subtract, op1=Alu.mult)
                    nc.vector.tensor_tensor(out=r0[:], in0=r0[:], in1=tmp1[:],
                                            op=Alu.subtract)

                t_t = main.tile([P, MCOL], f32)
                nc.vector.tensor_scalar(out=t_t[:], in0=obs_t[:],
                                        scalar1=wmin[:], scalar2=None,
                                        op0=Alu.subtract)
                q0 = main.tile([P, MCOL], f32)
                nc.vector.tensor_scalar(out=q0[:], in0=t_t[:], scalar1=r0[:],
                                        scalar2=None, op0=Alu.mult)
                pp = main.tile([P, MCOL], f32)
                nc.vector.tensor_scalar(out=pp[:], in0=q0[:], scalar1=dg[:],
                                        scalar2=None, op0=Alu.mult)
                ee = main.tile([P, MCOL], f32)
                nc.vector.tensor_tensor(out=ee[:], in0=t_t[:], in1=pp[:],
                                        op=Alu.subtract)
                pos = main.tile([P, MCOL], f32)
                nc.vector.scalar_tensor_tensor(out=pos[:], in0=ee[:],
                                               scalar=r0[:], in1=q0[:],
                                               op0=Alu.mult, op1=Alu.add)
                nc.vector.tensor_scalar(out=pos[:], in0=pos[:],
                                        scalar1=float(N - 1), scalar2=float(N - 1),
                                        op0=Alu.mult, op1=Alu.min)
                nc.vector.tensor_scalar(out=pos[:], in0=pos[:],
                                        scalar1=0.0, scalar2=None, op0=Alu.max)

                yy = main.tile([P, MCOL], f32)
                nc.vector.tensor_tensor(out=yy[:], in0=pos[:], in1=base_f[:],
                                        op=Alu.subtract)

                # ---- 8-tap hat filter: out = sum_k relu(1-|y-k|) * G[..k] ----
                H = main.tile([P, MCOL], f32)
                a_t = main.tile([P, MCOL], f32)
                w_t = main.tile([P, MCOL], f32)
                m_t = main.tile([P, MCOL], f32)
                if SKIP_SEL:
                    H = main.tile([P, MCOL], f32)
                    nc.vector.tensor_copy(out=H[:], in_=G[:, :, 0])
                    nc.sync.dma_start(out=out.ap(), in_=H[:])
                    continue
                negk = main.tile([P, WIN], f32)
                for k in range(WIN):
                    nc.vector.memset(negk[:, k:k + 1], -float(k))
                for k in range(WIN):
                    nc.scalar.activation(out=a_t[:], in_=yy[:],
                                         func=mybir.ActivationFunctionType.Abs,
                                         bias=negk[:, k:k + 1], scale=1.0)
                    nc.scalar.activation(out=w_t[:], in_=a_t[:],
                                         func=mybir.ActivationFunctionType.Relu,
                                         bias=1.0, scale=-1.0)
                    if k == 0:
                        nc.vector.tensor_tensor(out=H[:], in0=w_t[:],
                                                in1=G[:, :, 0], op=Alu.mult)
                    else:
                        nc.vector.tensor_tensor(out=m_t[:], in0=w_t[:],
                                                in1=G[:, :, k], op=Alu.mult)
                        nc.vector.tensor_tensor(out=H[:], in0=H[:], in1=m_t[:],
                                                op=Alu.add)

                nc.sync.dma_start(out=out.ap(), in_=H[:])

    nc.compile()
    return nc


def _get_nc():
    if "nc" not in _cache:
        _cache["nc"] = _build()
    return _cache["nc"]


def kernel(high_res_flux, high_res_wavelength, observed_wavelength):
    from concourse.bass_utils import run_bass_kernel_spmd

    if V4:
        try:
            return kernel_v4(high_res_flux, high_res_wavelength,
                             observed_wavelength)
        except RuntimeError:
            pass  # packing overflow: fall through

    if V3:
        try:
            return kernel_v3(high_res_flux, high_res_wavelength,
                             observed_wavelength)
        except RuntimeError:
            pass  # packing overflow: fall through to v2/v1 path

    if V2:
        try:
            return kernel_v2(high_res_flux, high_res_wavelength,
                             observed_wavelength)
        except RuntimeError:
            pass  # packing overflow: fall through to v1 path

    nc = _get_nc()
    high_res_flux = np.ascontiguousarray(high_res_flux, dtype=np.float32)
    high_res_wavelength = np.ascontiguousarray(high_res_wavelength,
                                               dtype=np.float32)
    observed_wavelength = np.ascontiguousarray(observed_wavelength,
                                               dtype=np.float32)

    in_maps = []
    for c in range(NUM_CORES):
        rows = slice(c * B_LOC, (c + 1) * B_LOC)
        in_maps.append({
            "flux": high_res_flux[rows].reshape(FLAT),
            "wav": high_res_wavelength[rows].reshape(P, WAV_COL),
            "obs": observed_wavelength[rows].reshape(P, MCOL),
        })

    res = run_bass_kernel_spmd(nc, in_maps, list(range(NUM_CORES)))
    full = np.empty((B, M), dtype=np.float32)
    for c in range(NUM_CORES):
        full[c * B_LOC:(c + 1) * B_LOC] = res.results[c]["out"].reshape(B_LOC, M)
    return full


def _build_v3(repeat=1, nch=None, dve_taps=None, fp32r=None, fp16=None):
    """v3: host-computed window bases/slot positions; device does a chunked
    multi-offset indirect gather of 16-float windows plus a 16-tap hat
    select accumulated in PSUM."""
    import concourse.bass as bass
    import concourse.bacc as bacc
    import concourse.mybir as mybir
    from concourse import tile
    from concourse.masks import make_identity

    nch = NCH3 if nch is None else nch
    dve_taps = DVE_TAPS if dve_taps is None else dve_taps
    fp32r = FP32R if fp32r is None else fp32r
    fp16 = FP16 if fp16 is None else fp16

    f32 = mybir.dt.float32
    f32r = mybir.dt.float32r
    f16 = mybir.dt.float16
    gdt = f16 if fp16 else f32
    i32 = mybir.dt.int32
    Alu = mybir.AluOpType
    Act = mybir.ActivationFunctionType

    assert NWCOL3 % nch == 0
    CW = NWCOL3 // nch              # window cols per chunk
    SCW = CW * R3                   # slot cols per chunk

    nc = bacc.Bacc("TRN2", target_bir_lowering=False, debug=False,
                   num_devices=NUM_CORES)
    flux = nc.dram_tensor("flux", [FLAT], gdt, kind="ExternalInput")
    offs_d = nc.dram_tensor("offs", [P, NWCOL3], i32, kind="ExternalInput")
    y_d = nc.dram_tensor("y", [P, SC3], f32, kind="ExternalInput")
    out_d = nc.dram_tensor("out", [P, SC3], f32, kind="ExternalOutput")

    flux2d = flux.ap().rearrange("(a b) -> a b", b=1)

    with tile.TileContext(nc) as tc:
        with (
            tc.tile_pool(name="main", bufs=1) as main,
            tc.tile_pool(name="gp", bufs=2) as gp,
            tc.tile_pool(name="mp", bufs=8) as mp,
            tc.tile_pool(name="ps", bufs=2, space="PSUM") as ps,
        ):
            ident = main.tile([P, P], gdt)
            make_identity(nc, ident[:])
            nident = main.tile([P, P], gdt)
            nc.vector.tensor_scalar(out=nident[:], in0=ident[:],
                                    scalar1=-1.0, scalar2=None, op0=Alu.mult)
            negk = main.tile([P, WIN3], f32)
            for k in range(WIN3):
                nc.vector.memset(negk[:, k:k + 1], -float(k))

            for _rep in range(repeat):
                offs_t = main.tile([P, NWCOL3], i32, tag="offs")
                y_t = main.tile([P, SC3], f32, tag="y")
                nc.sync.dma_start(out=offs_t[:], in_=offs_d.ap())
                nc.sync.dma_start(out=y_t[:], in_=y_d.ap())

                for ch in range(nch):
                    G = gp.tile([P, CW, WIN3], gdt, tag="G")
                    nc.gpsimd.indirect_dma_start(
                        out=G[:, :, :], out_offset=None, in_=flux2d,
                        in_offset=bass.IndirectOffsetOnAxis(
                            ap=offs_t[:, ch * CW:(ch + 1) * CW], axis=0))
                    ys = y_t[:, ch * SCW:(ch + 1) * SCW]
                    acc = ps.tile([P, SCW], f32, tag="acc")
                    for k in range(WIN3):
                        m_t = mp.tile([P, SCW], gdt, tag="m")
                        if k < dve_taps:
                            # DVE path: a=|y-k|; w_neg=min(a,1)-1; m=-w*G
                            a_t = mp.tile([P, SCW], f32, tag="a")
                            nc.vector.tensor_scalar(
                                out=a_t[:], in0=ys, scalar1=float(k),
                                scalar2=0.0, op0=Alu.subtract, op1=Alu.abs_max)
                            nc.vector.tensor_scalar(
                                out=a_t[:], in0=a_t[:], scalar1=1.0,
                                scalar2=1.0, op0=Alu.min, op1=Alu.subtract)
                            nc.vector.tensor_tensor(
                                out=m_t[:].rearrange("p (c r) -> p c r", r=R3),
                                in0=a_t[:].rearrange("p (c r) -> p c r", r=R3),
                                in1=G[:, :, k].to_broadcast([P, CW, R3]),
                                op=Alu.mult)
                            lhs = nident
                        else:
                            a_t = mp.tile([P, SCW], f32, tag="a")
                            w_t = mp.tile([P, SCW], gdt, tag="w")
                            nc.scalar.activation(out=a_t[:], in_=ys,
                                                 func=Act.Abs,
                                                 bias=negk[:, k:k + 1],
                                                 scale=1.0)
                            nc.scalar.activation(out=w_t[:], in_=a_t[:],
                                                 func=Act.Relu,
                                                 bias=1.0, scale=-1.0)
                            nc.vector.tensor_tensor(
                                out=m_t[:].rearrange("p (c r) -> p c r", r=R3),
                                in0=w_t[:].rearrange("p (c r) -> p c r", r=R3),
                                in1=G[:, :, k].to_broadcast([P, CW, R3]),
                                op=Alu.mult)
                            lhs = ident
                        if fp32r and not fp16:
                            nc.tensor.matmul(out=acc[:],
                                             lhsT=lhs[:].bitcast(f32r),
                                             rhs=m_t[:].bitcast(f32r),
                                             start=(k == 0),
                                             stop=(k == WIN3 - 1))
                        else:
                            nc.tensor.matmul(out=acc[:], lhsT=lhs[:],
                                             rhs=m_t[:], start=(k == 0),
                                             stop=(k == WIN3 - 1))
                    H = mp.tile([P, SCW], f32, tag="H")
                    nc.vector.tensor_copy(out=H[:], in_=acc[:])
                    nc.sync.dma_start(
                        out=out_d.ap()[:, ch * SCW:(ch + 1) * SCW],
                        in_=H[:])

    nc.compile()
    return nc


def _build_v4(repeat=1, nch=None, dve_taps=None):
    """v4: exact-fit slot bands. Per chunk the gather columns are
    [CWD doubles | CWS singles]; slots are [CWD r0 | CWD r1 | CWS]."""
    import concourse.bass as bass
    import concourse.bacc as bacc
    import concourse.mybir as mybir
    from concourse import tile
    from concourse.masks import make_identity

    nch = NCH4 if nch is None else nch
    dve_taps = DVE_TAPS4 if dve_taps is None else dve_taps

    f32 = mybir.dt.float32
    f16 = mybir.dt.float16
    i32 = mybir.dt.int32
    Alu = mybir.AluOpType
    Act = mybir.ActivationFunctionType

    assert ND_PP % nch == 0 and NS_PP % nch == 0
    CWD = ND_PP // nch
    CWS = NS_PP // nch
    CG = CWD + CWS                  # gather cols per chunk
    SCW = 2 * CWD + CWS             # slot cols per chunk

    nc = bacc.Bacc("TRN2", target_bir_lowering=False, debug=False,
                   num_devices=NUM_CORES, num_swdge_queues=4)
    flux = nc.dram_tensor("flux", [FLAT], f16, kind="ExternalInput")
    offs_d = nc.dram_tensor("offs", [P, nch * CG], i32, kind="ExternalInput")
    y_d = nc.dram_tensor("y", [P, nch * SCW], f32, kind="ExternalInput")
    out_d = nc.dram_tensor("out", [P, nch * SCW], f32, kind="ExternalOutput")

    flux2d = flux.ap().rearrange("(a b) -> a b", b=1)

    with tile.TileContext(nc) as tc:
        with (
            tc.tile_pool(name="main", bufs=1) as main,
            tc.tile_pool(name="gp", bufs=2) as gp,
            tc.tile_pool(name="mp", bufs=8) as mp,
            tc.tile_pool(name="ps", bufs=2, space="PSUM") as ps,
        ):
            ident = main.tile([P, P], f16)
            make_identity(nc, ident[:])
            nident = main.tile([P, P], f16)
            nc.vector.tensor_scalar(out=nident[:], in0=ident[:],
                                    scalar1=-1.0, scalar2=None, op0=Alu.mult)
            negk = main.tile([P, WIN3], f32)
            for k in range(WIN3):
                nc.vector.memset(negk[:, k:k + 1], -float(k))

            for _rep in range(repeat):
                offs_t = main.tile([P, nch * CG], i32, tag="offs")
                y_t = main.tile([P, nch * SCW], f32, tag="y")
                nc.sync.dma_start(out=offs_t[:], in_=offs_d.ap())
                nc.sync.dma_start(out=y_t[:], in_=y_d.ap())

                for ch in range(nch):
                    G = gp.tile([P, CG, WIN3], f16, tag="G")
                    for j in range(CG):
                        gi = nc.gpsimd.indirect_dma_start(
                            out=G[:, j, :], out_offset=None, in_=flux2d,
                            in_offset=bass.IndirectOffsetOnAxis(
                                ap=offs_t[:, ch * CG + j:ch * CG + j + 1],
                                axis=0))
                        q = j % 4
                        if q:
                            gi.ins.queue = f"qPoolDynamic{q}"
                    ys = y_t[:, ch * SCW:(ch + 1) * SCW]
                    acc = ps.tile([P, SCW], f32, tag="acc")
                    for k in range(WIN3):
                        m_t = mp.tile([P, SCW], f16, tag="m")
                        if k < dve_taps:
                            a_t = mp.tile([P, SCW], f32, tag="a")
                            nc.vector.tensor_scalar(
                                out=a_t[:], in0=ys, scalar1=float(k),
                                scalar2=0.0, op0=Alu.subtract, op1=Alu.abs_max)
                            nc.vector.tensor_scalar(
                                out=a_t[:], in0=a_t[:], scalar1=1.0,
                                scalar2=1.0, op0=Alu.min, op1=Alu.subtract)
                            wsrc = a_t
                            lhs = nident
                        else:
                            a_t = mp.tile([P, SCW], f32, tag="a")
                            w_t = mp.tile([P, SCW], f16, tag="w")
                            nc.scalar.activation(out=a_t[:], in_=ys,
                                                 func=Act.Abs,
                                                 bias=negk[:, k:k + 1],
                                                 scale=1.0)
                            nc.scalar.activation(out=w_t[:], in_=a_t[:],
                                                 func=Act.Relu,
                                                 bias=1.0, scale=-1.0)
                            wsrc = w_t
                            lhs = ident
                        nc.vector.tensor_tensor(
                            out=m_t[:, 0:CWD], in0=wsrc[:, 0:CWD],
                            in1=G[:, 0:CWD, k], op=Alu.mult)
                        nc.vector.tensor_tensor(
                            out=m_t[:, CWD:2 * CWD], in0=wsrc[:, CWD:2 * CWD],
                            in1=G[:, 0:CWD, k], op=Alu.mult)
                        nc.vector.tensor_tensor(
                            out=m_t[:, 2 * CWD:], in0=wsrc[:, 2 * CWD:],
                            in1=G[:, CWD:CG, k], op=Alu.mult)
                        nc.tensor.matmul(out=acc[:], lhsT=lhs[:],
                                         rhs=m_t[:], start=(k == 0),
                                         stop=(k == WIN3 - 1))
                    H = mp.tile([P, SCW], f32, tag="H")
                    nc.vector.tensor_copy(out=H[:], in_=acc[:])
                    nc.sync.dma_start(
                        out=out_d.ap()[:, ch * SCW:(ch + 1) * SCW],
                        in_=H[:])

    nc.compile()
    return nc


def _pack_v4(flux, wav, obs, nch=None):
    """Exact-fit packing. Returns offs [NC,P,nch*CG], y [NC,P,nch*SCW],
    slotmap [B,16,nch*SCW] (M = trash)."""
    nch = NCH4 if nch is None else nch
    CWD = ND_PP // nch
    CWS = NS_PP // nch
    CG = CWD + CWS
    SCW = 2 * CWD + CWS

    wmin = np.float32(wav.min())
    wmax = np.float32(wav.max())
    pos = (obs - wmin) / np.float32(wmax - wmin) * np.float32(N - 1)
    pos = np.clip(pos, np.float32(0.0), np.float32(N - 1))
    i0 = np.floor(pos).astype(np.int64)

    offs = np.zeros((NUM_CORES, P, nch * CG), dtype=np.int32)
    yv = np.full((NUM_CORES, P, nch * SCW), -5.0, dtype=np.float32)
    slotmap = np.full((B, 16, nch * SCW), M, dtype=np.int32)

    for b in range(B):
        order = np.argsort(i0[b], kind="stable")
        s = i0[b][order]
        dbl_base = []; dbl_m = []       # (base, [m0, m1])
        sgl_base = []; sgl_m = []
        j = 0
        while j < M:
            base = s[j]
            if j + 1 < M and s[j + 1] - base <= SPAN3:
                dbl_base.append(min(base, N - WIN3))
                dbl_m.append((order[j], order[j + 1]))
                j += 2
            else:
                sgl_base.append(min(base, N - WIN3))
                sgl_m.append(order[j])
                j += 1
        nd, ns = len(dbl_base), len(sgl_base)
        if nd > ND_PP * 16 or ns > NS_PP * 16:
            raise RuntimeError("v4 packing overflow")
        core, brow = divmod(b, B_LOC)
        pbase = 16 * brow
        rowoff = brow * N
        for d in range(nd):
            lane, col = d % 16, d // 16
            ch, c = divmod(col, CWD)
            offs[core, pbase + lane, ch * CG + c] = rowoff + dbl_base[d]
            p0 = pos[b][dbl_m[d][0]]
            p1 = pos[b][dbl_m[d][1]]
            yv[core, pbase + lane, ch * SCW + c] = \
                np.float32(np.float64(p0) - dbl_base[d])
            yv[core, pbase + lane, ch * SCW + CWD + c] = \
                np.float32(np.float64(p1) - dbl_base[d])
            slotmap[b, lane, ch * SCW + c] = dbl_m[d][0]
            slotmap[b, lane, ch * SCW + CWD + c] = dbl_m[d][1]
        for g in range(ns):
            lane, col = g % 16, g // 16
            ch, c = divmod(col, CWS)
            offs[core, pbase + lane, ch * CG + CWD + c] = rowoff + sgl_base[g]
            yv[core, pbase + lane, ch * SCW + 2 * CWD + c] = \
                np.float32(np.float64(pos[b][sgl_m[g]]) - sgl_base[g])
            slotmap[b, lane, ch * SCW + 2 * CWD + c] = sgl_m[g]
    return offs, yv, slotmap


def kernel_v4(high_res_flux, high_res_wavelength, observed_wavelength):
    from concourse.bass_utils import run_bass_kernel_spmd

    if "nc4" not in _cache:
        _cache["nc4"] = _build_v4()
    nc = _cache["nc4"]

    flux = np.ascontiguousarray(high_res_flux, dtype=np.float32)
    wav = np.ascontiguousarray(high_res_wavelength, dtype=np.float32)
    obs = np.ascontiguousarray(observed_wavelength, dtype=np.float32)

    offs, yv, slotmap = _pack_v4(flux, wav, obs)
    fl_dev = flux.astype(np.float16)
    in_maps = []
    for c in range(NUM_CORES):
        rows = slice(c * B_LOC, (c + 1) * B_LOC)
        in_maps.append({
            "flux": fl_dev[rows].reshape(FLAT),
            "offs": offs[c],
            "y": yv[c],
        })
    _cache["nc_used"] = nc
    _cache["in_maps"] = in_maps
    res = run_bass_kernel_spmd(nc, in_maps, list(range(NUM_CORES)))
    return _unpack_v4([r["out"] for r in res.results], slotmap)


def _unpack_v4(outs, slotmap):
    full = np.empty((B, M + 1), dtype=np.float32)
    for c in range(NUM_CORES):
        o = outs[c]
        for bb in range(B_LOC):
            b = c * B_LOC + bb
            for lane in range(16):
                full[b, slotmap[b, lane]] = o[16 * bb + lane]
    return full[:, :M]


def _pack_v3(flux, wav, obs):
    """Host packing for v3. Returns (offs [NCORES,P,NWCOL3] i32,
    y [NCORES,P,SC3] f32, slotmap [B, NW_ROW, R3] int64 output indices)."""
    wmin = np.float32(wav.min())
    wmax = np.float32(wav.max())
    t = obs - wmin
    d = np.float32(wmax - wmin)
    q = t / d
    pos = q * np.float32(N - 1)
    pos = np.clip(pos, np.float32(0.0), np.float32(N - 1))
    i0 = np.floor(pos).astype(np.int64)

    offs = np.zeros((NUM_CORES, P, NWCOL3), dtype=np.int32)
    yv = np.zeros((NUM_CORES, P, SC3), dtype=np.float32)
    slotmap = np.zeros((B, NW_ROW, R3), dtype=np.int64)

    for b in range(B):
        order = np.argsort(i0[b], kind="stable")
        s = i0[b][order]
        # greedy pack: window starts at s[j]; extend while span<=SPAN3, <R3
        bases = np.empty(NW_ROW, dtype=np.int64)
        mem = np.empty((NW_ROW, R3), dtype=np.int64)  # sorted-index members
        nw = 0
        j = 0
        while j < M:
            base = s[j]
            k = j + 1
            while k < M and s[k] - base <= SPAN3 and (k - j) < R3:
                k += 1
            if nw >= NW_ROW:
                raise RuntimeError("v3 packing overflow")
            bases[nw] = min(base, N - WIN3)
            for r in range(R3):
                mem[nw, r] = min(j + r, k - 1)
            nw += 1
            j = k
        # pad with copies of window 0
        bases[nw:] = bases[0]
        mem[nw:] = mem[0]
        mm = order[mem[:NW_ROW]]                  # [NW_ROW, R3] output m idx
        slotmap[b] = mm
        ywin = (pos[b][mm].astype(np.float64)
                - bases[:NW_ROW, None]).astype(np.float32)
        core, brow = divmod(b, B_LOC)
        # window w -> partition 16*brow + w%16, col (w//16); slot col c*R3+r
        part = 16 * brow + (np.arange(NW_ROW) % 16)
        col = np.arange(NW_ROW) // 16
        offs[core, part, col] = (brow * N + bases[:NW_ROW]).astype(np.int32)
        for r in range(R3):
            yv[core, part, col * R3 + r] = ywin[:, r]
    return offs, yv, slotmap, pos


def kernel_v3(high_res_flux, high_res_wavelength, observed_wavelength):
    from concourse.bass_utils import run_bass_kernel_spmd

    if "nc3" not in _cache:
        _cache["nc3"] = _build_v3()
    nc = _cache["nc3"]

    flux = np.ascontiguousarray(high_res_flux, dtype=np.float32)
    wav = np.ascontiguousarray(high_res_wavelength, dtype=np.float32)
    obs = np.ascontiguousarray(observed_wavelength, dtype=np.float32)

    offs, yv, slotmap, _pos = _pack_v3(flux, wav, obs)

    fl_dev = flux.astype(np.float16) if FP16 else flux
    in_maps = []
    for c in range(NUM_CORES):
        rows = slice(c * B_LOC, (c + 1) * B_LOC)
        in_maps.append({
            "flux": fl_dev[rows].reshape(FLAT),
            "offs": offs[c],
            "y": yv[c],
        })
    _cache["nc_used"] = nc
    _cache["in_maps"] = in_maps
    res = run_bass_kernel_spmd(nc, in_maps, list(range(NUM_CORES)))
    return _unpack_v3(res.results, slotmap)


def _unpack_v3(results, slotmap):
    full = np.empty((B, M), dtype=np.float32)
    for c in range(NUM_CORES):
        o = results[c]["out"]                     # [P, SC3]
        for bb in range(B_LOC):
            b = c * B_LOC + bb
            w = np.arange(NW_ROW)
            part = 16 * bb + (w % 16)
            col = w // 16
            for r in range(R3):
                full[b, slotmap[b, :, r]] = o[part, col * R3 + r]
    return full


def _build_v2(repeat=1):
    """Packed-window variant: outputs pre-sorted/grouped on host so each
    indirect-DMA window (WINW floats) serves up to R_SLOTS outputs."""
    import concourse.bass as bass
    import concourse.bacc as bacc
    import concourse.mybir as mybir
    import concourse.bass_isa as bass_isa
    from concourse import tile

    f32 = mybir.dt.float32
    i32 = mybir.dt.int32
    Alu = mybir.AluOpType

    nc = bacc.Bacc("TRN2", target_bir_lowering=False, debug=False,
                   num_devices=NUM_CORES)
    flux = nc.dram_tensor("flux", [FLAT], f32, kind="ExternalInput")
    wav = nc.dram_tensor("wav", [P, WAV_COL], f32, kind="ExternalInput")
    obs = nc.dram_tensor("obs", [P, MCOL2], f32, kind="ExternalInput")
    out = nc.dram_tensor("out", [P, MCOL2], f32, kind="ExternalOutput")

    flux2d = flux.ap().rearrange("(a b) -> a b", b=1)

    with tile.TileContext(nc) as tc:
        with (
            tc.tile_pool(name="wavp", bufs=2) as wavp,
            tc.tile_pool(name="main", bufs=1) as main,
            tc.tile_pool(name="gp", bufs=2) as gp,
            tc.tile_pool(name="mp", bufs=4) as mp,
            tc.tile_pool(name="ps", bufs=2, space="PSUM") as ps,
            tc.tile_pool(name="dram", bufs=1, space="DRAM") as dram,
        ):
            from concourse.masks import make_identity
            ident = main.tile([P, P], f32)
            make_identity(nc, ident[:])
            for _rep in range(repeat):
                cc_in = dram.tile([P, 2], f32)
                cc_out = dram.tile([P, 2], f32, addr_space="Shared")
                obs_t = main.tile([P, MCOL2], f32)
                nc.sync.dma_start(out=obs_t[:], in_=obs.ap())

                # ---- Phase A: local min/max (same as v1) ----
                mins = main.tile([P, WCH], f32)
                maxs = main.tile([P, WCH], f32)
                cw = WAV_COL // WCH
                for c in range(WCH):
                    wt = wavp.tile([P, cw], f32, tag="wav")
                    nc.sync.dma_start(out=wt[:], in_=wav.ap()[:, c * cw:(c + 1) * cw])
                    nc.vector.tensor_reduce(out=mins[:, c:c + 1], in_=wt[:],
                                            axis=mybir.AxisListType.X, op=Alu.min)
                    nc.vector.tensor_reduce(out=maxs[:, c:c + 1], in_=wt[:],
                                            axis=mybir.AxisListType.X, op=Alu.max)
                partial = main.tile([P, 2], f32)
                nmn = main.tile([P, 1], f32)
                nc.vector.tensor_reduce(out=nmn[:], in_=mins[:],
                                        axis=mybir.AxisListType.X, op=Alu.min)
                nc.vector.tensor_scalar(out=partial[:, 0:1], in0=nmn[:],
                                        scalar1=-1.0, scalar2=None, op0=Alu.mult)
                nc.vector.tensor_reduce(out=partial[:, 1:2], in_=maxs[:],
                                        axis=mybir.AxisListType.X, op=Alu.max)
                loc = main.tile([P, 2], f32)
                nc.gpsimd.partition_all_reduce(out_ap=loc[:], in_ap=partial[:],
                                               channels=P,
                                               reduce_op=bass_isa.ReduceOp.max)

                # ---- collective (overlaps gather) ----
                nc.sync.dma_start(out=cc_in[:], in_=loc[:])
                nc.gpsimd.collective_compute(
                    "AllReduce", Alu.max,
                    replica_groups=[list(range(NUM_CORES))],
                    ins=[cc_in.opt()], outs=[cc_out.opt()],
                )
                glob = main.tile([P, 2], f32)
                nc.sync.dma_start(out=glob[:], in_=cc_out[:])

                # ---- local estimate -> per-window base ----
                wmin_e = main.tile([P, 1], f32)
                nc.vector.tensor_scalar(out=wmin_e[:], in0=loc[:, 0:1],
                                        scalar1=-1.0, scalar2=None, op0=Alu.mult)
                d_e = main.tile([P, 1], f32)
                nc.vector.tensor_tensor(out=d_e[:], in0=loc[:, 1:2], in1=wmin_e[:],
                                        op=Alu.subtract)
                r_e = main.tile([P, 1], f32)
                nc.vector.reciprocal(out=r_e[:], in_=d_e[:])
                s_e = main.tile([P, 1], f32)
                nc.vector.tensor_scalar(out=s_e[:], in0=r_e[:],
                                        scalar1=float(N - 1), scalar2=None,
                                        op0=Alu.mult)
                pos_e = main.tile([P, MCOL2], f32)
                nc.vector.tensor_scalar(out=pos_e[:], in0=obs_t[:],
                                        scalar1=wmin_e[:], scalar2=s_e[:],
                                        op0=Alu.subtract, op1=Alu.mult)
                nc.vector.tensor_scalar(out=pos_e[:], in0=pos_e[:],
                                        scalar1=float(N - 1), scalar2=0.0,
                                        op0=Alu.min, op1=Alu.max)
                # per-window base = min over R_SLOTS slots, minus margin
                bwin = main.tile([P, NWINCOL], f32)
                nc.vector.tensor_reduce(
                    out=bwin[:],
                    in_=pos_e[:].rearrange("p (w r) -> p w r", r=R_SLOTS),
                    axis=mybir.AxisListType.X, op=Alu.min)
                bwin_i = main.tile([P, NWINCOL], i32)
                nc.vector.tensor_copy(out=bwin_i[:], in_=bwin[:])
                nc.vector.tensor_scalar(out=bwin_i[:], in0=bwin_i[:],
                                        scalar1=BASE_SHIFT, scalar2=None,
                                        op0=Alu.subtract)
                nc.vector.tensor_scalar(out=bwin_i[:], in0=bwin_i[:],
                                        scalar1=N - WINW, scalar2=0,
                                        op0=Alu.min, op1=Alu.max)
                bwin_f = main.tile([P, NWINCOL], f32)
                nc.vector.tensor_copy(out=bwin_f[:], in_=bwin_i[:])

                rowb = main.tile([P, 1], i32)
                nc.gpsimd.iota(out=rowb[:], pattern=[[0, 1]], base=0,
                               channel_multiplier=1)
                nc.vector.tensor_scalar(out=rowb[:], in0=rowb[:],
                                        scalar1=4, scalar2=None,
                                        op0=Alu.logical_shift_right)
                nc.vector.tensor_scalar(out=rowb[:], in0=rowb[:],
                                        scalar1=N, scalar2=None, op0=Alu.mult)
                rowb_f = main.tile([P, 1], f32)
                nc.vector.tensor_copy(out=rowb_f[:], in_=rowb[:])
                offs_f = main.tile([P, NWINCOL], f32)
                nc.vector.tensor_scalar(out=offs_f[:], in0=bwin_f[:],
                                        scalar1=rowb_f[:], scalar2=None,
                                        op0=Alu.add)
                offs = main.tile([P, NWINCOL], i32)
                nc.vector.tensor_copy(out=offs[:], in_=offs_f[:])

                # ---- exact global pos (bit-exact) ----
                wmin = main.tile([P, 1], f32)
                nc.vector.tensor_scalar(out=wmin[:], in0=glob[:, 0:1],
                                        scalar1=-1.0, scalar2=None, op0=Alu.mult)
                dg = main.tile([P, 1], f32)
                nc.vector.tensor_tensor(out=dg[:], in0=glob[:, 1:2], in1=wmin[:],
                                        op=Alu.subtract)
                r0 = main.tile([P, 1], f32)
                nc.vector.reciprocal(out=r0[:], in_=dg[:])
                tmp1 = main.tile([P, 1], f32)
                for _ in range(2):
                    nc.vector.tensor_tensor(out=tmp1[:], in0=dg[:], in1=r0[:],
                                            op=Alu.mult)
                    nc.vector.scalar_tensor_tensor(out=tmp1[:], in0=tmp1[:],
                                                   scalar=1.0, in1=r0[:],
                                                   op0=Alu.subtract, op1=Alu.mult)
                    nc.vector.tensor_tensor(out=r0[:], in0=r0[:], in1=tmp1[:],
                                            op=Alu.subtract)
                t_t = main.tile([P, MCOL2], f32)
                nc.vector.tensor_scalar(out=t_t[:], in0=obs_t[:],
                                        scalar1=wmin[:], scalar2=None,
                                        op0=Alu.subtract)
                q0 = main.tile([P, MCOL2], f32)
                nc.vector.tensor_scalar(out=q0[:], in0=t_t[:], scalar1=r0[:],
                                        scalar2=None, op0=Alu.mult)
                pp = main.tile([P, MCOL2], f32)
                nc.vector.tensor_scalar(out=pp[:], in0=q0[:], scalar1=dg[:],
                                        scalar2=None, op0=Alu.mult)
                ee = main.tile([P, MCOL2], f32)
                nc.vector.tensor_tensor(out=ee[:], in0=t_t[:], in1=pp[:],
                                        op=Alu.subtract)
                pos = main.tile([P, MCOL2], f32)
                nc.vector.scalar_tensor_tensor(out=pos[:], in0=ee[:],
                                               scalar=r0[:], in1=q0[:],
                                               op0=Alu.mult, op1=Alu.add)
                nc.vector.tensor_scalar(out=pos[:], in0=pos[:],
                                        scalar1=float(N - 1), scalar2=float(N - 1),
                                        op0=Alu.mult, op1=Alu.min)
                nc.vector.tensor_scalar(out=pos[:], in0=pos[:],
                                        scalar1=0.0, scalar2=None, op0=Alu.max)

                # y = pos - base (base broadcast over R_SLOTS)
                yy = main.tile([P, MCOL2], f32)
                nc.vector.tensor_tensor(
                    out=yy[:].rearrange("p (w r) -> p w r", r=R_SLOTS),
                    in0=pos[:].rearrange("p (w r) -> p w r", r=R_SLOTS),
                    in1=bwin_f[:].to_broadcast([P, NWINCOL, R_SLOTS]),
                    op=Alu.subtract)

                # ---- chunked gather + WINW-tap hat select ----
                H = main.tile([P, MCOL2], f32)
                negk = main.tile([P, WINW], f32)
                for k in range(WINW):
                    nc.vector.memset(negk[:, k:k + 1], -float(k))
                NCH = 4
                wch = NWINCOL // NCH           # windows per chunk
                sch = wch * R_SLOTS            # slot-cols per chunk
                for ci in range(NCH):
                    G = gp.tile([P, wch, WINW], f32, tag="G")
                    for j in range(wch):
                        nc.gpsimd.indirect_dma_start(
                            out=G[:, j, :],
                            out_offset=None,
                            in_=flux2d,
                            in_offset=bass.IndirectOffsetOnAxis(
                                ap=offs[:, ci * wch + j:ci * wch + j + 1], axis=0),
                        )
                    a_t = main.tile([P, sch], f32, tag="a_t")
                    w_t = main.tile([P, sch], f32, tag="w_t")
                    ys = yy[:, ci * sch:(ci + 1) * sch]
                    Hs = H[:, ci * sch:(ci + 1) * sch]
                    acc = ps.tile([P, sch], f32, tag="acc")
                    for k in range(WINW):
                        nc.scalar.activation(out=a_t[:], in_=ys,
                                             func=mybir.ActivationFunctionType.Abs,
                                             bias=negk[:, k:k + 1], scale=1.0)
                        nc.scalar.activation(out=w_t[:], in_=a_t[:],
                                             func=mybir.ActivationFunctionType.Relu,
                                             bias=1.0, scale=-1.0)
                        gk = G[:, :, k].to_broadcast([P, wch, R_SLOTS])
                        w3 = w_t[:].rearrange("p (w r) -> p w r", r=R_SLOTS)
                        m_t = mp.tile([P, sch], f32, tag="m_t")
                        nc.vector.tensor_tensor(
                            out=m_t[:].rearrange("p (w r) -> p w r", r=R_SLOTS),
                            in0=w3, in1=gk, op=Alu.mult)
                        nc.tensor.matmul(out=acc[:], lhsT=ident[:], rhs=m_t[:],
                                         start=(k == 0), stop=(k == WINW - 1))
                    nc.vector.tensor_copy(out=Hs, in_=acc[:])

                nc.sync.dma_start(out=out.ap(), in_=H[:])

    nc.compile()
    return nc


def _pack_rows(obs_full, wav_full):
    """Host packing: per row, sort outputs by obs and greedily pack into
    windows of <= R_SLOTS outputs spanning <= SPAN_MAX estimated positions.
    Returns (obs_packed [B, NWIN_ROW*R_SLOTS], slotmap [B, NWIN_ROW*R_SLOTS])."""
    wmin = float(wav_full.min())
    wmax = float(wav_full.max())
    scale = (N - 1) / (wmax - wmin)
    nslots = NWIN_ROW * R_SLOTS
    obs_packed = np.empty((B, nslots), dtype=np.float32)
    slotmap = np.zeros((B, nslots), dtype=np.int32)
    for b in range(B):
        row = obs_full[b]
        order = np.argsort(row, kind="stable")
        g = np.clip((row[order].astype(np.float64) - wmin) * scale, 0, N - 1)
        g = g.astype(np.int64)
        # greedy: window start s covers outputs s .. reach[s]-1
        limit = np.searchsorted(g, g + SPAN_MAX, side="right")
        reach = np.minimum(limit, np.arange(M) + R_SLOTS)
        starts = []
        s = 0
        while s < M:
            starts.append(s)
            s = reach[s]
        nw = len(starts)
        if nw > NWIN_ROW:
            raise RuntimeError(f"packing overflow: {nw} > {NWIN_ROW}")
        starts = np.asarray(starts, dtype=np.int64)
        ends = np.empty_like(starts)
        ends[:-1] = starts[1:]
        ends[-1] = M
        # fill slots: window w slot r -> output order[min(starts[w]+r, ends[w]-1)]
        idx = starts[:, None] + np.arange(R_SLOTS)[None, :]
        idx = np.minimum(idx, (ends - 1)[:, None])
        sm = order[idx]                      # [nw, R_SLOTS] original m indices
        smf = np.empty((NWIN_ROW, R_SLOTS), dtype=np.int64)
        smf[:nw] = sm
        smf[nw:] = sm[0, 0]                  # pad windows duplicate a real output
        slotmap[b] = smf.reshape(-1)
        obs_packed[b] = row[smf.reshape(-1)]
    return obs_packed, slotmap


def kernel_v2(high_res_flux, high_res_wavelength, observed_wavelength):
    from concourse.bass_utils import run_bass_kernel_spmd

    if "nc2" not in _cache:
        _cache["nc2"] = _build_v2()
    nc = _cache["nc2"]

    flux = np.ascontiguousarray(high_res_flux, dtype=np.float32)
    wav = np.ascontiguousarray(high_res_wavelength, dtype=np.float32)
    obs = np.ascontiguousarray(observed_wavelength, dtype=np.float32)

    obs_packed, slotmap = _pack_rows(obs, wav)

    in_maps = []
    for c in range(NUM_CORES):
        rows = slice(c * B_LOC, (c + 1) * B_LOC)
        in_maps.append({
            "flux": flux[rows].reshape(FLAT),
            "wav": wav[rows].reshape(P, WAV_COL),
            "obs": obs_packed[rows].reshape(P, MCOL2),
        })
    _cache["nc_used"] = nc
    _cache["in_maps"] = in_maps
    res = run_bass_kernel_spmd(nc, in_maps, list(range(NUM_CORES)))
    # (kernel_v2 keeps its own unpack below)
    full = np.empty((B, M), dtype=np.float32)
    for c in range(NUM_CORES):
        o = res.results[c]["out"].reshape(B_LOC, NWIN_ROW * R_SLOTS)
        for bb in range(B_LOC):
            b = c * B_LOC + bb
            full[b, slotmap[b]] = o[bb]
    return full



# revision 28
# speedup vs baseline: 1.0479x; 1.0479x over previous
"""Trainium2 Bass kernel for nn_DownsamplingLayer (grid_sample-degenerate 1-D lerp).

out[b, m] = lerp(flux[b, :], pos[b, m]) where
pos = clip((obs - wmin) / (wmax - wmin) * (N-1), 0, N-1),
wmin/wmax are global min/max over high_res_wavelength.

Strategy (8 NeuronCores, pure data-parallel over batch, 8 rows/core):
 - Phase A: stream wavelength shard, DVE min/max reduce + gpsimd
   partition_all_reduce -> core-LOCAL (negmin, max).
 - Speculative gather: positions estimated from LOCAL min/max; one
   indirect-DMA per output column gathers an 8-float window per partition
   (window absorbs local-vs-global estimate error; P(miss) ~ 1e-12 for
   the spec's random fills).
 - Overlapped collective AllReduce(max) of (-min, max) gives the exact
   global wmin/wmax; exact positions use a Markstein-corrected reciprocal
   so pos is bit-identical to IEEE f32 division.
 - 8-tap hat-filter (DVE + ACT relu) turns the gathered window into the
   exact linear interpolation.
"""
import sys

for _p in ("/opt/trn_rl_repo",):
    if _p not in sys.path:
        sys.path.insert(0, _p)

import numpy as np

B, N, M = 64, 262144, 16384
NUM_CORES = 8
B_LOC = B // NUM_CORES          # 8 rows per core
P = 128                         # SBUF partitions
MCOL = B_LOC * M // P           # 1024 obs columns per partition
WAV_COL = B_LOC * N // P        # 16384 wavelength columns per partition
FLAT = B_LOC * N                # flux flat length per core
WIN = 8                         # gathered window (f32 elems per output)
BASE_SHIFT = 3                  # window starts at floor(pos_est) - 3
WCH = 4                         # wavelength chunks for min/max streaming
NGATHER = None                  # debug: limit gather instruction count
NQUEUES = 1                     # SWDGE queues for the gather (1..4)

# ---- v3 (host-packed greedy windows + 16-tap hat select) ----
V3 = False                      # multi-offset indirect DMA broken on HW                       # use v3 path in kernel()
NW_ROW = 10560                  # windows per row (measured max 10341; mult of 16)
NWCOL3 = NW_ROW // 16           # 660 window columns per partition
R3 = 2                          # output slots per window
SC3 = NWCOL3 * R3               # 1320 slot columns per partition
SPAN3 = 14                      # max i0 spread within a window
WIN3 = 16                       # gathered window width (f32)
NCH3 = 3                        # gather/select chunks (660 % 3 == 0)
DVE_TAPS = 6                    # taps 0..DVE_TAPS-1 computed on DVE, rest ACT
FP32R = False                   # fp32r accumulate matmuls
FP16 = True                     # flux/windows/products in fp16

# ---- v4 (exact-fit bands: doubles get 2 slots, singles 1) ----
# Correct on HW (rel err 2.7e-4) but its 672 per-column SWDGE gathers are
# likely slower than v2's 268; local wall-clock noise cannot resolve it.
V4 = False
ND_PP = 390                     # doubles window cols per partition (max 378)
NS_PP = 282                     # singles window cols per partition (max 270)
GC4 = ND_PP + NS_PP             # gather cols per partition = 672
SC4 = 2 * ND_PP + NS_PP        # slot cols per partition = 1062
NCH4 = 3                        # chunks; ND_PP, NS_PP divisible by NCH4
DVE_TAPS4 = 0                   # abs_max tensor_scalar fails walrus ISA check

# ---- v2 (packed-window) parameters ----
V2 = True                       # use packed-window path in kernel()
R_SLOTS = 3                     # output slots per window
WINW = 48                      # gathered window width (f32)
SPAN_MAX = 32                  # host packing span budget (<= WINW - 16)
NWIN_ROW = 6912                # padded windows per row (multiple of 16)
NWINCOL = NWIN_ROW * B_LOC // P      # windows per partition = 448
MCOL2 = NWINCOL * R_SLOTS            # obs' columns per partition = 1792
SKIP_CC = False                 # debug: skip collective
SKIP_A = False                  # debug: skip min/max phase
SKIP_SEL = False                # debug: skip select phase

_cache = {}


def _build(repeat=1):
    import concourse.bass as bass
    import concourse.bacc as bacc
    import concourse.mybir as mybir
    import concourse.bass_isa as bass_isa
    from concourse import tile

    f32 = mybir.dt.float32
    i32 = mybir.dt.int32
    Alu = mybir.AluOpType

    nc = bacc.Bacc("TRN2", target_bir_lowering=False, debug=False,
                   num_devices=NUM_CORES, num_swdge_queues=NQUEUES)
    flux = nc.dram_tensor("flux", [FLAT], f32, kind="ExternalInput")
    wav = nc.dram_tensor("wav", [P, WAV_COL], f32, kind="ExternalInput")
    obs = nc.dram_tensor("obs", [P, MCOL], f32, kind="ExternalInput")
    out = nc.dram_tensor("out", [P, MCOL], f32, kind="ExternalOutput")

    flux2d = flux.ap().rearrange("(a b) -> a b", b=1)

    with tile.TileContext(nc) as tc:
        with (
            tc.tile_pool(name="wavp", bufs=2) as wavp,
            tc.tile_pool(name="main", bufs=1) as main,
            tc.tile_pool(name="dram", bufs=1, space="DRAM") as dram,
        ):
            for _rep in range(repeat):
                cc_in = dram.tile([P, 2], f32)
                cc_out = dram.tile([P, 2], f32, addr_space="Shared")
                obs_t = main.tile([P, MCOL], f32)
                nc.sync.dma_start(out=obs_t[:], in_=obs.ap())

                # ---- Phase A: local min/max over the wavelength shard ----
                mins = main.tile([P, WCH], f32)
                maxs = main.tile([P, WCH], f32)
                cw = WAV_COL // WCH
                for c in range(0 if SKIP_A else WCH):
                    wt = wavp.tile([P, cw], f32, tag="wav")
                    nc.sync.dma_start(out=wt[:], in_=wav.ap()[:, c * cw:(c + 1) * cw])
                    nc.vector.tensor_reduce(out=mins[:, c:c + 1], in_=wt[:],
                                            axis=mybir.AxisListType.X, op=Alu.min)
                    nc.vector.tensor_reduce(out=maxs[:, c:c + 1], in_=wt[:],
                                            axis=mybir.AxisListType.X, op=Alu.max)
                partial = main.tile([P, 2], f32)
                if SKIP_A:
                    nc.vector.memset(partial[:, 0:1], -1e-6)
                    nc.vector.memset(partial[:, 1:2], 1.0 - 1e-6)
                # col0 = -(min over chunks), col1 = max over chunks
                nmn = main.tile([P, 1], f32)
                if not SKIP_A:
                    nc.vector.tensor_reduce(out=nmn[:], in_=mins[:],
                                        axis=mybir.AxisListType.X, op=Alu.min)
                    nc.vector.tensor_scalar(out=partial[:, 0:1], in0=nmn[:],
                                            scalar1=-1.0, scalar2=None, op0=Alu.mult)
                    nc.vector.tensor_reduce(out=partial[:, 1:2], in_=maxs[:],
                                            axis=mybir.AxisListType.X, op=Alu.max)

                # local all-partition reduce (max of (-min, max) = (-gmin, gmax))
                loc = main.tile([P, 2], f32)
                nc.gpsimd.partition_all_reduce(out_ap=loc[:], in_ap=partial[:],
                                               channels=P,
                                               reduce_op=bass_isa.ReduceOp.max)

                # ---- cross-core collective (overlaps the gather below) ----
                glob = main.tile([P, 2], f32)
                if SKIP_CC:
                    nc.vector.tensor_copy(out=glob[:], in_=loc[:])
                else:
                    nc.sync.dma_start(out=cc_in[:], in_=loc[:])
                    nc.gpsimd.collective_compute(
                        "AllReduce", Alu.max,
                        replica_groups=[list(range(NUM_CORES))],
                        ins=[cc_in.opt()], outs=[cc_out.opt()],
                    )
                    nc.sync.dma_start(out=glob[:], in_=cc_out[:])

                # ---- local estimate -> window bases + gather offsets ----
                wmin_e = main.tile([P, 1], f32)
                nc.vector.tensor_scalar(out=wmin_e[:], in0=loc[:, 0:1],
                                        scalar1=-1.0, scalar2=None, op0=Alu.mult)
                d_e = main.tile([P, 1], f32)
                nc.vector.tensor_tensor(out=d_e[:], in0=loc[:, 1:2], in1=wmin_e[:],
                                        op=Alu.subtract)
                r_e = main.tile([P, 1], f32)
                nc.vector.reciprocal(out=r_e[:], in_=d_e[:])
                s_e = main.tile([P, 1], f32)
                nc.vector.tensor_scalar(out=s_e[:], in0=r_e[:],
                                        scalar1=float(N - 1), scalar2=None,
                                        op0=Alu.mult)
                pos_e = main.tile([P, MCOL], f32)
                nc.vector.tensor_scalar(out=pos_e[:], in0=obs_t[:],
                                        scalar1=wmin_e[:], scalar2=s_e[:],
                                        op0=Alu.subtract, op1=Alu.mult)
                nc.vector.tensor_scalar(out=pos_e[:], in0=pos_e[:],
                                        scalar1=float(N - 1), scalar2=0.0,
                                        op0=Alu.min, op1=Alu.max)
                base_i = main.tile([P, MCOL], i32)
                nc.vector.tensor_copy(out=base_i[:], in_=pos_e[:])
                nc.vector.tensor_scalar(out=base_i[:], in0=base_i[:],
                                        scalar1=BASE_SHIFT, scalar2=None,
                                        op0=Alu.subtract)
                nc.vector.tensor_scalar(out=base_i[:], in0=base_i[:],
                                        scalar1=N - WIN, scalar2=0,
                                        op0=Alu.min, op1=Alu.max)
                base_f = main.tile([P, MCOL], f32)
                nc.vector.tensor_copy(out=base_f[:], in_=base_i[:])

                # rowbase[p] = (p // 16) * N  (f32 add is exact: values < 2^24)
                rowb = main.tile([P, 1], i32)
                nc.gpsimd.iota(out=rowb[:], pattern=[[0, 1]], base=0,
                               channel_multiplier=1)
                nc.vector.tensor_scalar(out=rowb[:], in0=rowb[:],
                                        scalar1=4, scalar2=None,
                                        op0=Alu.logical_shift_right)
                nc.vector.tensor_scalar(out=rowb[:], in0=rowb[:],
                                        scalar1=N, scalar2=None, op0=Alu.mult)
                rowb_f = main.tile([P, 1], f32)
                nc.vector.tensor_copy(out=rowb_f[:], in_=rowb[:])
                offs_f = main.tile([P, MCOL], f32)
                nc.vector.tensor_scalar(out=offs_f[:], in0=base_f[:],
                                        scalar1=rowb_f[:], scalar2=None,
                                        op0=Alu.add)
                offs = main.tile([P, MCOL], i32)
                nc.vector.tensor_copy(out=offs[:], in_=offs_f[:])

                # ---- speculative window gather: one indirect DMA per column ----
                G = main.tile([P, MCOL, WIN], f32)
                ng = MCOL if NGATHER is None else NGATHER
                if ng < MCOL:
                    nc.vector.memset(G[:, ng:, :], 0.0)
                for j in range(ng):
                    gi = nc.gpsimd.indirect_dma_start(
                        out=G[:, j, :],
                        out_offset=None,
                        in_=flux2d,
                        in_offset=bass.IndirectOffsetOnAxis(ap=offs[:, j:j + 1],
                                                            axis=0),
                    )
                    if NQUEUES > 1:
                        q = j % NQUEUES
                        if q:
                            gi.ins.queue = f"qPoolDynamic{q}"


                # ---- exact global pos (bit-exact vs IEEE f32 reference) ----
                wmin = main.tile([P, 1], f32)
                nc.vector.tensor_scalar(out=wmin[:], in0=glob[:, 0:1],
                                        scalar1=-1.0, scalar2=None, op0=Alu.mult)
                dg = main.tile([P, 1], f32)
                nc.vector.tensor_tensor(out=dg[:], in0=glob[:, 1:2], in1=wmin[:],
                                        op=Alu.subtract)
                r0 = main.tile([P, 1], f32)
                nc.vector.reciprocal(out=r0[:], in_=dg[:])
                # two Newton iterations: r <- r*(2 - d*r)
                tmp1 = main.tile([P, 1], f32)
                for _ in range(2):
                    nc.vector.tensor_tensor(out=tmp1[:], in0=dg[:], in1=r0[:],
                                            op=Alu.mult)
                    nc.vector.scalar_tensor_tensor(out=tmp1[:], in0=tmp1[:],
                                                   scalar=1.0, in1=r0[:],
                                                   op0=Alu.subtract, op1=Alu.mult)
                    nc.vector.tensor_tensor(out=r0[:], in0=r0[:], in1=tmp1[:],
                                            op=Alu.subtract)

                t_t = main.tile([P, MCOL], f32)
                nc.vector.tensor_scalar(out=t_t[:], in0=obs_t[:],
                                        scalar1=wmin[:], scalar2=None,
                                        op0=Alu.subtract)
                q0 = main.tile([P, MCOL], f32)
                nc.vector.tensor_scalar(out=q0[:], in0=t_t[:], scalar1=r0[:],
                                        scalar2=None, op0=Alu.mult)
                pp = main.tile([P, MCOL], f32)
                nc.vector.tensor_scalar(out=pp[:], in0=q0[:], scalar1=dg[:],
                                        scalar2=None, op0=Alu.mult)
                ee = main.tile([P, MCOL], f32)
                nc.vector.tensor_tensor(out=ee[:], in0=t_t[:], in1=pp[:],
                                        op=Alu.subtract)
                pos = main.tile([P, MCOL], f32)
                nc.vector.scalar_tensor_tensor(out=pos[:], in0=ee[:],
                                               scalar=r0[:], in1=q0[:],
                                               op0=Alu.mult, op1=Alu.add)
                nc.vector.tensor_scalar(out=pos[:], in0=pos[:],
                                        scalar1=float(N - 1), scalar2=float(N - 1),
                                        op0=Alu.mult, op1=Alu.min)
                nc.vector.tensor_scalar(out=pos[:], in0=pos[:],
                                        scalar1=0.0, scalar2=None, op0=Alu.max)

                yy = main.tile([P, MCOL], f32)
                nc.vector.tensor_tensor(out=yy[:], in0=pos[:], in1=base_f[:],
                                        op=Alu.subtract)

                # ---- 8-tap hat filter: out = sum_k relu(1-|y-k|) * G[..k] ----
                H = main.tile([P, MCOL], f32)
                a_t = main.tile([P, MCOL], f32)
                w_t = main.tile([P, MCOL], f32)
                m_t = main.tile([P, MCOL], f32)
                if SKIP_SEL:
                    H = main.tile([P, MCOL], f32)
                    nc.vector.tensor_copy(out=H[:], in_=G[:, :, 0])
                    nc.sync.dma_start(out=out.ap(), in_=H[:])
                    continue
                negk = main.tile([P, WIN], f32)
                for k in range(WIN):
                    nc.vector.memset(negk[:, k:k + 1], -float(k))
                for k in range(WIN):
                    nc.scalar.activation(out=a_t[:], in_=yy[:],
                                         func=mybir.ActivationFunctionType.Abs,
                                         bias=negk[:, k:k + 1], scale=1.0)
                    nc.scalar.activation(out=w_t[:], in_=a_t[:],
                                         func=mybir.ActivationFunctionType.Relu,
                                         bias=1.0, scale=-1.0)
                    if k == 0:
                        nc.vector.tensor_tensor(out=H[:], in0=w_t[:],
                                                in1=G[:, :, 0], op=Alu.mult)
                    else:
                        nc.vector.tensor_tensor(out=m_t[:], in0=w_t[:],
                                                in1=G[:, :, k], op=Alu.mult)
                        nc.vector.tensor_tensor(out=H[:], in0=H[:], in1=m_t[:],
                                                op=Alu.add)

                nc.sync.dma_start(out=out.ap(), in_=H[:])

    nc.compile()
    return nc


def _get_nc():
    if "nc" not in _cache:
        _cache["nc"] = _build()
    return _cache["nc"]


def kernel(high_res_flux, high_res_wavelength, observed_wavelength):
    from concourse.bass_utils import run_bass_kernel_spmd

    if V4:
        try:
            return kernel_v4(high_res_flux, high_res_wavelength,
                             observed_wavelength)
        except RuntimeError:
            pass  # packing overflow: fall through

    if V3:
        try:
            return kernel_v3(high_res_flux, high_res_wavelength,
                             observed_wavelength)
        except RuntimeError:
            pass  # packing overflow: fall through to v2/v1 path

    if V2:
        try:
            return kernel_v2(high_res_flux, high_res_wavelength,
                             observed_wavelength)
        except RuntimeError:
            pass  # packing overflow: fall through to v1 path

    nc = _get_nc()
    high_res_flux = np.ascontiguousarray(high_res_flux, dtype=np.float32)
    high_res_wavelength = np.ascontiguousarray(high_res_wavelength,
                                               dtype=np.float32)
    observed_wavelength = np.ascontiguousarray(observed_wavelength,
                                               dtype=np.float32)

    in_maps = []
    for c in range(NUM_CORES):
        rows = slice(c * B_LOC, (c + 1) * B_LOC)
        in_maps.append({
            "flux": high_res_flux[rows].reshape(FLAT),
            "wav": high_res_wavelength[rows].reshape(P, WAV_COL),
            "obs": observed_wavelength[rows].reshape(P, MCOL),
        })

    res = run_bass_kernel_spmd(nc, in_maps, list(range(NUM_CORES)))
    full = np.empty((B, M), dtype=np.float32)
    for c in range(NUM_CORES):
        full[c * B_LOC:(c + 1) * B_LOC] = res.results[c]["out"].reshape(B_LOC, M)
    return full


def _build_v3(repeat=1, nch=None, dve_taps=None, fp32r=None, fp16=None):
    """v3: host-computed window bases/slot positions; device does a chunked
    multi-offset indirect gather of 16-float windows plus a 16-tap hat
    select accumulated in PSUM."""
    import concourse.bass as bass
    import concourse.bacc as bacc
    import concourse.mybir as mybir
    from concourse import tile
    from concourse.masks import make_identity

    nch = NCH3 if nch is None else nch
    dve_taps = DVE_TAPS if dve_taps is None else dve_taps
    fp32r = FP32R if fp32r is None else fp32r
    fp16 = FP16 if fp16 is None else fp16

    f32 = mybir.dt.float32
    f32r = mybir.dt.float32r
    f16 = mybir.dt.float16
    gdt = f16 if fp16 else f32
    i32 = mybir.dt.int32
    Alu = mybir.AluOpType
    Act = mybir.ActivationFunctionType

    assert NWCOL3 % nch == 0
    CW = NWCOL3 // nch              # window cols per chunk
    SCW = CW * R3                   # slot cols per chunk

    nc = bacc.Bacc("TRN2", target_bir_lowering=False, debug=False,
                   num_devices=NUM_CORES)
    flux = nc.dram_tensor("flux", [FLAT], gdt, kind="ExternalInput")
    offs_d = nc.dram_tensor("offs", [P, NWCOL3], i32, kind="ExternalInput")
    y_d = nc.dram_tensor("y", [P, SC3], f32, kind="ExternalInput")
    out_d = nc.dram_tensor("out", [P, SC3], f32, kind="ExternalOutput")

    flux2d = flux.ap().rearrange("(a b) -> a b", b=1)

    with tile.TileContext(nc) as tc:
        with (
            tc.tile_pool(name="main", bufs=1) as main,
            tc.tile_pool(name="gp", bufs=2) as gp,
            tc.tile_pool(name="mp", bufs=8) as mp,
            tc.tile_pool(name="ps", bufs=2, space="PSUM") as ps,
        ):
            ident = main.tile([P, P], gdt)
            make_identity(nc, ident[:])
            nident = main.tile([P, P], gdt)
            nc.vector.tensor_scalar(out=nident[:], in0=ident[:],
                                    scalar1=-1.0, scalar2=None, op0=Alu.mult)
            negk = main.tile([P, WIN3], f32)
            for k in range(WIN3):
                nc.vector.memset(negk[:, k:k + 1], -float(k))

            for _rep in range(repeat):
                offs_t = main.tile([P, NWCOL3], i32, tag="offs")
                y_t = main.tile([P, SC3], f32, tag="y")
                nc.sync.dma_start(out=offs_t[:], in_=offs_d.ap())
                nc.sync.dma_start(out=y_t[:], in_=y_d.ap())

                for ch in range(nch):
                    G = gp.tile([P, CW, WIN3], gdt, tag="G")
                    nc.gpsimd.indirect_dma_start(
                        out=G[:, :, :], out_offset=None, in_=flux2d,
                        in_offset=bass.IndirectOffsetOnAxis(
                            ap=offs_t[:, ch * CW:(ch + 1) * CW], axis=0))
                    ys = y_t[:, ch * SCW:(ch + 1) * SCW]
                    acc = ps.tile([P, SCW], f32, tag="acc")
                    for k in range(WIN3):
                        m_t = mp.tile([P, SCW], gdt, tag="m")
                        if k < dve_taps:
                            # DVE path: a=|y-k|; w_neg=min(a,1)-1; m=-w*G
                            a_t = mp.tile([P, SCW], f32, tag="a")
                            nc.vector.tensor_scalar(
                                out=a_t[:], in0=ys, scalar1=float(k),
                                scalar2=0.0, op0=Alu.subtract, op1=Alu.abs_max)
                            nc.vector.tensor_scalar(
                                out=a_t[:], in0=a_t[:], scalar1=1.0,
                                scalar2=1.0, op0=Alu.min, op1=Alu.subtract)
                            nc.vector.tensor_tensor(
                                out=m_t[:].rearrange("p (c r) -> p c r", r=R3),
                                in0=a_t[:].rearrange("p (c r) -> p c r", r=R3),
                                in1=G[:, :, k].to_broadcast([P, CW, R3]),
                                op=Alu.mult)
                            lhs = nident
                        else:
                            a_t = mp.tile([P, SCW], f32, tag="a")
                            w_t = mp.tile([P, SCW], gdt, tag="w")
                            nc.scalar.activation(out=a_t[:], in_=ys,
                                                 func=Act.Abs,
                                                 bias=negk[:, k:k + 1],
                                                 scale=1.0)
                            nc.scalar.activation(out=w_t[:], in_=a_t[:],
                                                 func=Act.Relu,
                                                 bias=1.0, scale=-1.0)
                            nc.vector.tensor_tensor(
                                out=m_t[:].rearrange("p (c r) -> p c r", r=R3),
                                in0=w_t[:].rearrange("p (c r) -> p c r", r=R3),
                                in1=G[:, :, k].to_broadcast([P, CW, R3]),
                                op=Alu.mult)
                            lhs = ident
                        if fp32r and not fp16:
                            nc.tensor.matmul(out=acc[:],
                                             lhsT=lhs[:].bitcast(f32r),
                                             rhs=m_t[:].bitcast(f32r),
                                             start=(k == 0),
                                             stop=(k == WIN3 - 1))
                        else:
                            nc.tensor.matmul(out=acc[:], lhsT=lhs[:],
                                             rhs=m_t[:], start=(k == 0),
                                             stop=(k == WIN3 - 1))
                    H = mp.tile([P, SCW], f32, tag="H")
                    nc.vector.tensor_copy(out=H[:], in_=acc[:])
                    nc.sync.dma_start(
                        out=out_d.ap()[:, ch * SCW:(ch + 1) * SCW],
                        in_=H[:])

    nc.compile()
    return nc


def _build_v4(repeat=1, nch=None, dve_taps=None):
    """v4: exact-fit slot bands. Per chunk the gather columns are
    [CWD doubles | CWS singles]; slots are [CWD r0 | CWD r1 | CWS]."""
    import concourse.bass as bass
    import concourse.bacc as bacc
    import concourse.mybir as mybir
    from concourse import tile
    from concourse.masks import make_identity

    nch = NCH4 if nch is None else nch
    dve_taps = DVE_TAPS4 if dve_taps is None else dve_taps

    f32 = mybir.dt.float32
    f16 = mybir.dt.float16
    i32 = mybir.dt.int32
    Alu = mybir.AluOpType
    Act = mybir.ActivationFunctionType

    assert ND_PP % nch == 0 and NS_PP % nch == 0
    CWD = ND_PP // nch
    CWS = NS_PP // nch
    CG = CWD + CWS                  # gather cols per chunk
    SCW = 2 * CWD + CWS             # slot cols per chunk

    nc = bacc.Bacc("TRN2", target_bir_lowering=False, debug=False,
                   num_devices=NUM_CORES, num_swdge_queues=4)
    flux = nc.dram_tensor("flux", [FLAT], f16, kind="ExternalInput")
    offs_d = nc.dram_tensor("offs", [P, nch * CG], i32, kind="ExternalInput")
    y_d = nc.dram_tensor("y", [P, nch * SCW], f32, kind="ExternalInput")
    out_d = nc.dram_tensor("out", [P, nch * SCW], f32, kind="ExternalOutput")

    flux2d = flux.ap().rearrange("(a b) -> a b", b=1)

    with tile.TileContext(nc) as tc:
        with (
            tc.tile_pool(name="main", bufs=1) as main,
            tc.tile_pool(name="gp", bufs=2) as gp,
            tc.tile_pool(name="mp", bufs=8) as mp,
            tc.tile_pool(name="ps", bufs=2, space="PSUM") as ps,
        ):
            ident = main.tile([P, P], f16)
            make_identity(nc, ident[:])
            nident = main.tile([P, P], f16)
            nc.vector.tensor_scalar(out=nident[:], in0=ident[:],
                                    scalar1=-1.0, scalar2=None, op0=Alu.mult)
            negk = main.tile([P, WIN3], f32)
            for k in range(WIN3):
                nc.vector.memset(negk[:, k:k + 1], -float(k))

            for _rep in range(repeat):
                offs_t = main.tile([P, nch * CG], i32, tag="offs")
                y_t = main.tile([P, nch * SCW], f32, tag="y")
                nc.sync.dma_start(out=offs_t[:], in_=offs_d.ap())
                nc.sync.dma_start(out=y_t[:], in_=y_d.ap())

                for ch in range(nch):
                    G = gp.tile([P, CG, WIN3], f16, tag="G")
                    for j in range(CG):
                        gi = nc.gpsimd.indirect_dma_start(
                            out=G[:, j, :], out_offset=None, in_=flux2d,
                            in_offset=bass.IndirectOffsetOnAxis(
                                ap=offs_t[:, ch * CG + j:ch * CG + j + 1],
                                axis=0))
                        q = j % 4
                        if q:
                            gi.ins.queue = f"qPoolDynamic{q}"
                    ys = y_t[:, ch * SCW:(ch + 1) * SCW]
                    acc = ps.tile([P, SCW], f32, tag="acc")
                    for k in range(WIN3):
                        m_t = mp.tile([P, SCW], f16, tag="m")
                        if k < dve_taps:
                            a_t = mp.tile([P, SCW], f32, tag="a")
                            nc.vector.tensor_scalar(
                                out=a_t[:], in0=ys, scalar1=float(k),
                                scalar2=0.0, op0=Alu.subtract, op1=Alu.abs_max)
                            nc.vector.tensor_scalar(
                                out=a_t[:], in0=a_t[:], scalar1=1.0,
                                scalar2=1.0, op0=Alu.min, op1=Alu.subtract)
                            wsrc = a_t
                            lhs = nident
                        else:
                            a_t = mp.tile([P, SCW], f32, tag="a")
                            w_t = mp.tile([P, SCW], f16, tag="w")
                            nc.scalar.activation(out=a_t[:], in_=ys,
                                                 func=Act.Abs,
                                                 bias=negk[:, k:k + 1],
                                                 scale=1.0)
                            nc.scalar.activation(out=w_t[:], in_=a_t[:],
                                                 func=Act.Relu,
                                                 bias=1.0, scale=-1.0)
                            wsrc = w_t
                            lhs = ident
                        nc.vector.tensor_tensor(
                            out=m_t[:, 0:CWD], in0=wsrc[:, 0:CWD],
                            in1=G[:, 0:CWD, k], op=Alu.mult)
                        nc.vector.tensor_tensor(
                            out=m_t[:, CWD:2 * CWD], in0=wsrc[:, CWD:2 * CWD],
                            in1=G[:, 0:CWD, k], op=Alu.mult)
                        nc.vector.tensor_tensor(
                            out=m_t[:, 2 * CWD:], in0=wsrc[:, 2 * CWD:],
                            in1=G[:, CWD:CG, k], op=Alu.mult)
                        nc.tensor.matmul(out=acc[:], lhsT=lhs[:],
                                         rhs=m_t[:], start=(k == 0),
                                         stop=(k == WIN3 - 1))
                    H = mp.tile([P, SCW], f32, tag="H")
                    nc.vector.tensor_copy(out=H[:], in_=acc[:])
                    nc.sync.dma_start(
                        out=out_d.ap()[:, ch * SCW:(ch + 1) * SCW],
                        in_=H[:])

    nc.compile()
    return nc


def _pack_v4(flux, wav, obs, nch=None):
    """Exact-fit packing. Returns offs [NC,P,nch*CG], y [NC,P,nch*SCW],
    slotmap [B,16,nch*SCW] (M = trash)."""
    nch = NCH4 if nch is None else nch
    CWD = ND_PP // nch
    CWS = NS_PP // nch
    CG = CWD + CWS
    SCW = 2 * CWD + CWS

    wmin = np.float32(wav.min())
    wmax = np.float32(wav.max())
    pos = (obs - wmin) / np.float32(wmax - wmin) * np.float32(N - 1)
    pos = np.clip(pos, np.float32(0.0), np.float32(N - 1))
    i0 = np.floor(pos).astype(np.int64)

    offs = np.zeros((NUM_CORES, P, nch * CG), dtype=np.int32)
    yv = np.full((NUM_CORES, P, nch * SCW), -5.0, dtype=np.float32)
    slotmap = np.full((B, 16, nch * SCW), M, dtype=np.int32)

    for b in range(B):
        order = np.argsort(i0[b], kind="stable")
        s = i0[b][order]
        dbl_base = []; dbl_m = []       # (base, [m0, m1])
        sgl_base = []; sgl_m = []
        j = 0
        while j < M:
            base = s[j]
            if j + 1 < M and s[j + 1] - base <= SPAN3:
                dbl_base.append(min(base, N - WIN3))
                dbl_m.append((order[j], order[j + 1]))
                j += 2
            else:
                sgl_base.append(min(base, N - WIN3))
                sgl_m.append(order[j])
                j += 1
        nd, ns = len(dbl_base), len(sgl_base)
        if nd > ND_PP * 16 or ns > NS_PP * 16:
            raise RuntimeError("v4 packing overflow")
        core, brow = divmod(b, B_LOC)
        pbase = 16 * brow
        rowoff = brow * N
        for d in range(nd):
            lane, col = d % 16, d // 16
            ch, c = divmod(col, CWD)
            offs[core, pbase + lane, ch * CG + c] = rowoff + dbl_base[d]
            p0 = pos[b][dbl_m[d][0]]
            p1 = pos[b][dbl_m[d][1]]
            yv[core, pbase + lane, ch * SCW + c] = \
                np.float32(np.float64(p0) - dbl_base[d])
            yv[core, pbase + lane, ch * SCW + CWD + c] = \
                np.float32(np.float64(p1) - dbl_base[d])
            slotmap[b, lane, ch * SCW + c] = dbl_m[d][0]
            slotmap[b, lane, ch * SCW + CWD + c] = dbl_m[d][1]
        for g in range(ns):
            lane, col = g % 16, g // 16
            ch, c = divmod(col, CWS)
            offs[core, pbase + lane, ch * CG + CWD + c] = rowoff + sgl_base[g]
            yv[core, pbase + lane, ch * SCW + 2 * CWD + c] = \
                np.float32(np.float64(pos[b][sgl_m[g]]) - sgl_base[g])
            slotmap[b, lane, ch * SCW + 2 * CWD + c] = sgl_m[g]
    return offs, yv, slotmap


def kernel_v4(high_res_flux, high_res_wavelength, observed_wavelength):
    from concourse.bass_utils import run_bass_kernel_spmd

    if "nc4" not in _cache:
        _cache["nc4"] = _build_v4()
    nc = _cache["nc4"]

    flux = np.ascontiguousarray(high_res_flux, dtype=np.float32)
    wav = np.ascontiguousarray(high_res_wavelength, dtype=np.float32)
    obs = np.ascontiguousarray(observed_wavelength, dtype=np.float32)

    offs, yv, slotmap = _pack_v4(flux, wav, obs)
    fl_dev = flux.astype(np.float16)
    in_maps = []
    for c in range(NUM_CORES):
        rows = slice(c * B_LOC, (c + 1) * B_LOC)
        in_maps.append({
            "flux": fl_dev[rows].reshape(FLAT),
            "offs": offs[c],
            "y": yv[c],
        })
    _cache["nc_used"] = nc
    _cache["in_maps"] = in_maps
    res = run_bass_kernel_spmd(nc, in_maps, list(range(NUM_CORES)))
    return _unpack_v4([r["out"] for r in res.results], slotmap)


def _unpack_v4(outs, slotmap):
    full = np.empty((B, M + 1), dtype=np.float32)
    for c in range(NUM_CORES):
        o = outs[c]
        for bb in range(B_LOC):
            b = c * B_LOC + bb
            for lane in range(16):
                full[b, slotmap[b, lane]] = o[16 * bb + lane]
    return full[:, :M]


def _pack_v3(flux, wav, obs):
    """Host packing for v3. Returns (offs [NCORES,P,NWCOL3] i32,
    y [NCORES,P,SC3] f32, slotmap [B, NW_ROW, R3] int64 output indices)."""
    wmin = np.float32(wav.min())
    wmax = np.float32(wav.max())
    t = obs - wmin
    d = np.float32(wmax - wmin)
    q = t / d
    pos = q * np.float32(N - 1)
    pos = np.clip(pos, np.float32(0.0), np.float32(N - 1))
    i0 = np.floor(pos).astype(np.int64)

    offs = np.zeros((NUM_CORES, P, NWCOL3), dtype=np.int32)
    yv = np.zeros((NUM_CORES, P, SC3), dtype=np.float32)
    slotmap = np.zeros((B, NW_ROW, R3), dtype=np.int64)

    for b in range(B):
        order = np.argsort(i0[b], kind="stable")
        s = i0[b][order]
        # greedy pack: window starts at s[j]; extend while span<=SPAN3, <R3
        bases = np.empty(NW_ROW, dtype=np.int64)
        mem = np.empty((NW_ROW, R3), dtype=np.int64)  # sorted-index members
        nw = 0
        j = 0
        while j < M:
            base = s[j]
            k = j + 1
            while k < M and s[k] - base <= SPAN3 and (k - j) < R3:
                k += 1
            if nw >= NW_ROW:
                raise RuntimeError("v3 packing overflow")
            bases[nw] = min(base, N - WIN3)
            for r in range(R3):
                mem[nw, r] = min(j + r, k - 1)
            nw += 1
            j = k
        # pad with copies of window 0
        bases[nw:] = bases[0]
        mem[nw:] = mem[0]
        mm = order[mem[:NW_ROW]]                  # [NW_ROW, R3] output m idx
        slotmap[b] = mm
        ywin = (pos[b][mm].astype(np.float64)
                - bases[:NW_ROW, None]).astype(np.float32)
        core, brow = divmod(b, B_LOC)
        # window w -> partition 16*brow + w%16, col (w//16); slot col c*R3+r
        part = 16 * brow + (np.arange(NW_ROW) % 16)
        col = np.arange(NW_ROW) // 16
        offs[core, part, col] = (brow * N + bases[:NW_ROW]).astype(np.int32)
        for r in range(R3):
            yv[core, part, col * R3 + r] = ywin[:, r]
    return offs, yv, slotmap, pos


def kernel_v3(high_res_flux, high_res_wavelength, observed_wavelength):
    from concourse.bass_utils import run_bass_kernel_spmd

    if "nc3" not in _cache:
        _cache["nc3"] = _build_v3()
    nc = _cache["nc3"]

    flux = np.ascontiguousarray(high_res_flux, dtype=np.float32)
    wav = np.ascontiguousarray(high_res_wavelength, dtype=np.float32)
    obs = np.ascontiguousarray(observed_wavelength, dtype=np.float32)

    offs, yv, slotmap, _pos = _pack_v3(flux, wav, obs)

    fl_dev = flux.astype(np.float16) if FP16 else flux
    in_maps = []
    for c in range(NUM_CORES):
        rows = slice(c * B_LOC, (c + 1) * B_LOC)
        in_maps.append({
            "flux": fl_dev[rows].reshape(FLAT),
            "offs": offs[c],
            "y": yv[c],
        })
    _cache["nc_used"] = nc
    _cache["in_maps"] = in_maps
    res = run_bass_kernel_spmd(nc, in_maps, list(range(NUM_CORES)))
    return _unpack_v3(res.results, slotmap)


def _unpack_v3(results, slotmap):
    full = np.empty((B, M), dtype=np.float32)
    for c in range(NUM_CORES):
        o = results[c]["out"]                     # [P, SC3]
        for bb in range(B_LOC):
            b = c * B_LOC + bb
            w = np.arange(NW_ROW)
            part = 16 * bb + (w % 16)
            col = w // 16
            for r in range(R3):
                full[b, slotmap[b, :, r]] = o[part, col * R3 + r]
    return full


def _build_v2(repeat=1):
    """Packed-window variant: outputs pre-sorted/grouped on host so each
    indirect-DMA window (WINW floats) serves up to R_SLOTS outputs."""
    import concourse.bass as bass
    import concourse.bacc as bacc
    import concourse.mybir as mybir
    import concourse.bass_isa as bass_isa
    from concourse import tile

    f32 = mybir.dt.float32
    i32 = mybir.dt.int32
    Alu = mybir.AluOpType

    nc = bacc.Bacc("TRN2", target_bir_lowering=False, debug=False,
                   num_devices=NUM_CORES)
    flux = nc.dram_tensor("flux", [FLAT], f32, kind="ExternalInput")
    wav = nc.dram_tensor("wav", [P, WAV_COL], f32, kind="ExternalInput")
    obs = nc.dram_tensor("obs", [P, MCOL2], f32, kind="ExternalInput")
    out = nc.dram_tensor("out", [P, MCOL2], f32, kind="ExternalOutput")

    flux2d = flux.ap().rearrange("(a b) -> a b", b=1)

    with tile.TileContext(nc) as tc:
        with (
            tc.tile_pool(name="wavp", bufs=2) as wavp,
            tc.tile_pool(name="main", bufs=1) as main,
            tc.tile_pool(name="gp", bufs=2) as gp,
            tc.tile_pool(name="mp", bufs=8) as mp,
            tc.tile_pool(name="ps", bufs=2, space="PSUM") as ps,
            tc.tile_pool(name="dram", bufs=1, space="DRAM") as dram,
        ):
            from concourse.masks import make_identity
            ident = main.tile([P, P], f32)
            make_identity(nc, ident[:])
            for _rep in range(repeat):
                cc_in = dram.tile([P, 2], f32)
                cc_out = dram.tile([P, 2], f32, addr_space="Shared")
                obs_t = main.tile([P, MCOL2], f32)
                nc.sync.dma_start(out=obs_t[:], in_=obs.ap())

                # ---- Phase A: local min/max (same as v1) ----
                mins = main.tile([P, WCH], f32)
                maxs = main.tile([P, WCH], f32)
                cw = WAV_COL // WCH
                for c in range(WCH):
                    wt = wavp.tile([P, cw], f32, tag="wav")
                    nc.sync.dma_start(out=wt[:], in_=wav.ap()[:, c * cw:(c + 1) * cw])
                    nc.vector.tensor_reduce(out=mins[:, c:c + 1], in_=wt[:],
                                            axis=mybir.AxisListType.X, op=Alu.min)
                    nc.vector.tensor_reduce(out=maxs[:, c:c + 1], in_=wt[:],
                                            axis=mybir.AxisListType.X, op=Alu.max)
                partial = main.tile([P, 2], f32)
                nmn = main.tile([P, 1], f32)
                nc.vector.tensor_reduce(out=nmn[:], in_=mins[:],
                                        axis=mybir.AxisListType.X, op=Alu.min)
                nc.vector.tensor_scalar(out=partial[:, 0:1], in0=nmn[:],
                                        scalar1=-1.0, scalar2=None, op0=Alu.mult)
                nc.vector.tensor_reduce(out=partial[:, 1:2], in_=maxs[:],
                                        axis=mybir.AxisListType.X, op=Alu.max)
                loc = main.tile([P, 2], f32)
                nc.gpsimd.partition_all_reduce(out_ap=loc[:], in_ap=partial[:],
                                               channels=P,
                                               reduce_op=bass_isa.ReduceOp.max)

                # ---- collective (overlaps gather) ----
                nc.sync.dma_start(out=cc_in[:], in_=loc[:])
                nc.gpsimd.collective_compute(
                    "AllReduce", Alu.max,
                    replica_groups=[list(range(NUM_CORES))],
                    ins=[cc_in.opt()], outs=[cc_out.opt()],
                )
                glob = main.tile([P, 2], f32)
                nc.sync.dma_start(out=glob[:], in_=cc_out[:])

                # ---- local estimate -> per-window base ----
                wmin_e = main.tile([P, 1], f32)
                nc.vector.tensor_scalar(out=wmin_e[:], in0=loc[:, 0:1],
                                        scalar1=-1.0, scalar2=None, op0=Alu.mult)
                d_e = main.tile([P, 1], f32)
                nc.vector.tensor_tensor(out=d_e[:], in0=loc[:, 1:2], in1=wmin_e[:],
                                        op=Alu.subtract)
                r_e = main.tile([P, 1], f32)
                nc.vector.reciprocal(out=r_e[:], in_=d_e[:])
                s_e = main.tile([P, 1], f32)
                nc.vector.tensor_scalar(out=s_e[:], in0=r_e[:],
                                        scalar1=float(N - 1), scalar2=None,
                                        op0=Alu.mult)
                pos_e = main.tile([P, MCOL2], f32)
                nc.vector.tensor_scalar(out=pos_e[:], in0=obs_t[:],
                                        scalar1=wmin_e[:], scalar2=s_e[:],
                                        op0=Alu.subtract, op1=Alu.mult)
                nc.vector.tensor_scalar(out=pos_e[:], in0=pos_e[:],
                                        scalar1=float(N - 1), scalar2=0.0,
                                        op0=Alu.min, op1=Alu.max)
                # per-window base = min over R_SLOTS slots, minus margin
                bwin = main.tile([P, NWINCOL], f32)
                nc.vector.tensor_reduce(
                    out=bwin[:],
                    in_=pos_e[:].rearrange("p (w r) -> p w r", r=R_SLOTS),
                    axis=mybir.AxisListType.X, op=Alu.min)
                bwin_i = main.tile([P, NWINCOL], i32)
                nc.vector.tensor_copy(out=bwin_i[:], in_=bwin[:])
                nc.vector.tensor_scalar(out=bwin_i[:], in0=bwin_i[:],
                                        scalar1=BASE_SHIFT, scalar2=None,
                                        op0=Alu.subtract)
                nc.vector.tensor_scalar(out=bwin_i[:], in0=bwin_i[:],
                                        scalar1=N - WINW, scalar2=0,
                                        op0=Alu.min, op1=Alu.max)
                bwin_f = main.tile([P, NWINCOL], f32)
                nc.vector.tensor_copy(out=bwin_f[:], in_=bwin_i[:])

                rowb = main.tile([P, 1], i32)
                nc.gpsimd.iota(out=rowb[:], pattern=[[0, 1]], base=0,
                               channel_multiplier=1)
                nc.vector.tensor_scalar(out=rowb[:], in0=rowb[:],
                                        scalar1=4, scalar2=None,
                                        op0=Alu.logical_shift_right)
                nc.vector.tensor_scalar(out=rowb[:], in0=rowb[:],
                                        scalar1=N, scalar2=None, op0=Alu.mult)
                rowb_f = main.tile([P, 1], f32)
                nc.vector.tensor_copy(out=rowb_f[:], in_=rowb[:])
                offs_f = main.tile([P, NWINCOL], f32)
                nc.vector.tensor_scalar(out=offs_f[:], in0=bwin_f[:],
                                        scalar1=rowb_f[:], scalar2=None,
                                        op0=Alu.add)
                offs = main.tile([P, NWINCOL], i32)
                nc.vector.tensor_copy(out=offs[:], in_=offs_f[:])

                # ---- exact global pos (bit-exact) ----
                wmin = main.tile([P, 1], f32)
                nc.vector.tensor_scalar(out=wmin[:], in0=glob[:, 0:1],
                                        scalar1=-1.0, scalar2=None, op0=Alu.mult)
                dg = main.tile([P, 1], f32)
                nc.vector.tensor_tensor(out=dg[:], in0=glob[:, 1:2], in1=wmin[:],
                                        op=Alu.subtract)
                r0 = main.tile([P, 1], f32)
                nc.vector.reciprocal(out=r0[:], in_=dg[:])
                tmp1 = main.tile([P, 1], f32)
                for _ in range(2):
                    nc.vector.tensor_tensor(out=tmp1[:], in0=dg[:], in1=r0[:],
                                            op=Alu.mult)
                    nc.vector.scalar_tensor_tensor(out=tmp1[:], in0=tmp1[:],
                                                   scalar=1.0, in1=r0[:],
                                                   op0=Alu.subtract, op1=Alu.mult)
                    nc.vector.tensor_tensor(out=r0[:], in0=r0[:], in1=tmp1[:],
                                            op=Alu.subtract)
                t_t = main.tile([P, MCOL2], f32)
                nc.vector.tensor_scalar(out=t_t[:], in0=obs_t[:],
                                        scalar1=wmin[:], scalar2=None,
                                        op0=Alu.subtract)
                q0 = main.tile([P, MCOL2], f32)
                nc.vector.tensor_scalar(out=q0[:], in0=t_t[:], scalar1=r0[:],
                                        scalar2=None, op0=Alu.mult)
                pp = main.tile([P, MCOL2], f32)
                nc.vector.tensor_scalar(out=pp[:], in0=q0[:], scalar1=dg[:],
                                        scalar2=None, op0=Alu.mult)
                ee = main.tile([P, MCOL2], f32)
                nc.vector.tensor_tensor(out=ee[:], in0=t_t[:], in1=pp[:],
                                        op=Alu.subtract)
                pos = main.tile([P, MCOL2], f32)
                nc.vector.scalar_tensor_tensor(out=pos[:], in0=ee[:],
                                               scalar=r0[:], in1=q0[:],
                                               op0=Alu.mult, op1=Alu.add)
                nc.vector.tensor_scalar(out=pos[:], in0=pos[:],
                                        scalar1=float(N - 1), scalar2=float(N - 1),
                                        op0=Alu.mult, op1=Alu.min)
                nc.vector.tensor_scalar(out=pos[:], in0=pos[:],
                                        scalar1=0.0, scalar2=None, op0=Alu.max)

                # y = pos - base (base broadcast over R_SLOTS)
                yy = main.tile([P, MCOL2], f32)
                nc.vector.tensor_tensor(
                    out=yy[:].rearrange("p (w r) -> p w r", r=R_SLOTS),
                    in0=pos[:].rearrange("p (w r) -> p w r", r=R_SLOTS),
                    in1=bwin_f[:].to_broadcast([P, NWINCOL, R_SLOTS]),
                    op=Alu.subtract)

                # ---- chunked gather + WINW-tap hat select ----
                H = main.tile([P, MCOL2], f32)
                negk = main.tile([P, WINW], f32)
                for k in range(WINW):
                    nc.vector.memset(negk[:, k:k + 1], -float(k))
                NCH = 4
                wch = NWINCOL // NCH           # windows per chunk
                sch = wch * R_SLOTS            # slot-cols per chunk
                for ci in range(NCH):
                    G = gp.tile([P, wch, WINW], f32, tag="G")
                    for j in range(wch):
                        nc.gpsimd.indirect_dma_start(
                            out=G[:, j, :],
                            out_offset=None,
                            in_=flux2d,
                            in_offset=bass.IndirectOffsetOnAxis(
                                ap=offs[:, ci * wch + j:ci * wch + j + 1], axis=0),
                        )
                    ys = yy[:, ci * sch:(ci + 1) * sch]
                    Hs = H[:, ci * sch:(ci + 1) * sch]
                    acc = ps.tile([P, sch], f32, tag="acc")
                    for k in range(WINW):
                        a_t = mp.tile([P, sch], f32, tag="a_t")
                        w_t = mp.tile([P, sch], f32, tag="w_t")
                        nc.scalar.activation(out=a_t[:], in_=ys,
                                             func=mybir.ActivationFunctionType.Abs,
                                             bias=negk[:, k:k + 1], scale=1.0)
                        nc.scalar.activation(out=w_t[:], in_=a_t[:],
                                             func=mybir.ActivationFunctionType.Relu,
                                             bias=1.0, scale=-1.0)
                        gk = G[:, :, k].to_broadcast([P, wch, R_SLOTS])
                        w3 = w_t[:].rearrange("p (w r) -> p w r", r=R_SLOTS)
                        m_t = mp.tile([P, sch], f32, tag="m_t")
                        nc.vector.tensor_tensor(
                            out=m_t[:].rearrange("p (w r) -> p w r", r=R_SLOTS),
                            in0=w3, in1=gk, op=Alu.mult)
                        nc.tensor.matmul(out=acc[:], lhsT=ident[:], rhs=m_t[:],
                                         start=(k == 0), stop=(k == WINW - 1))
                    nc.vector.tensor_copy(out=Hs, in_=acc[:])

                nc.sync.dma_start(out=out.ap(), in_=H[:])

    nc.compile()
    return nc


def _pack_rows(obs_full, wav_full):
    """Host packing: per row, sort outputs by obs and greedily pack into
    windows of <= R_SLOTS outputs spanning <= SPAN_MAX estimated positions.
    Returns (obs_packed [B, NWIN_ROW*R_SLOTS], slotmap [B, NWIN_ROW*R_SLOTS])."""
    wmin = float(wav_full.min())
    wmax = float(wav_full.max())
    scale = (N - 1) / (wmax - wmin)
    nslots = NWIN_ROW * R_SLOTS
    obs_packed = np.empty((B, nslots), dtype=np.float32)
    slotmap = np.zeros((B, nslots), dtype=np.int32)
    for b in range(B):
        row = obs_full[b]
        order = np.argsort(row, kind="stable")
        g = np.clip((row[order].astype(np.float64) - wmin) * scale, 0, N - 1)
        g = g.astype(np.int64)
        # greedy: window start s covers outputs s .. reach[s]-1
        limit = np.searchsorted(g, g + SPAN_MAX, side="right")
        reach = np.minimum(limit, np.arange(M) + R_SLOTS)
        starts = []
        s = 0
        while s < M:
            starts.append(s)
            s = reach[s]
        nw = len(starts)
        if nw > NWIN_ROW:
            raise RuntimeError(f"packing overflow: {nw} > {NWIN_ROW}")
        starts = np.asarray(starts, dtype=np.int64)
        ends = np.empty_like(starts)
        ends[:-1] = starts[1:]
        ends[-1] = M
        # fill slots: window w slot r -> output order[min(starts[w]+r, ends[w]-1)]
        idx = starts[:, None] + np.arange(R_SLOTS)[None, :]
        idx = np.minimum(idx, (ends - 1)[:, None])
        sm = order[idx]                      # [nw, R_SLOTS] original m indices
        smf = np.empty((NWIN_ROW, R_SLOTS), dtype=np.int64)
        smf[:nw] = sm
        smf[nw:] = sm[0, 0]                  # pad windows duplicate a real output
        slotmap[b] = smf.reshape(-1)
        obs_packed[b] = row[smf.reshape(-1)]
    return obs_packed, slotmap


def kernel_v2(high_res_flux, high_res_wavelength, observed_wavelength):
    from concourse.bass_utils import run_bass_kernel_spmd

    if "nc2" not in _cache:
        _cache["nc2"] = _build_v2()
    nc = _cache["nc2"]

    flux = np.ascontiguousarray(high_res_flux, dtype=np.float32)
    wav = np.ascontiguousarray(high_res_wavelength, dtype=np.float32)
    obs = np.ascontiguousarray(observed_wavelength, dtype=np.float32)

    obs_packed, slotmap = _pack_rows(obs, wav)

    in_maps = []
    for c in range(NUM_CORES):
        rows = slice(c * B_LOC, (c + 1) * B_LOC)
        in_maps.append({
            "flux": flux[rows].reshape(FLAT),
            "wav": wav[rows].reshape(P, WAV_COL),
            "obs": obs_packed[rows].reshape(P, MCOL2),
        })
    _cache["nc_used"] = nc
    _cache["in_maps"] = in_maps
    res = run_bass_kernel_spmd(nc, in_maps, list(range(NUM_CORES)))
    # (kernel_v2 keeps its own unpack below)
    full = np.empty((B, M), dtype=np.float32)
    for c in range(NUM_CORES):
        o = res.results[c]["out"].reshape(B_LOC, NWIN_ROW * R_SLOTS)
        for bb in range(B_LOC):
            b = c * B_LOC + bb
            full[b, slotmap[b]] = o[bb]
    return full

